# revision 38
# baseline (speedup 1.0000x reference)
"""Trainium2 Bass kernel for nn_BatteryGNN (CGConv message-passing GNN).

Self-contained: takes full inputs, shards graph-data-parallel across 8
NeuronCores, runs a single SPMD NEFF (10 CGConv layers + pooling + MLP heads),
gathers per-core head outputs on the host.

Design (vs original baseline):
- Per-edge work restructured around a per-layer "U table":
  U[n] = h[n] @ [Wf_src | Ws_src] + [bf | bs]  (256 wide, f32),
  computed per owned 128-node block, all-gathered, then ONE dma_gather per
  edge chunk fetches U[src] rows directly in [edge, 256] pre-act layout.
  This kills the baseline's dst gather (halves the SWDGE gather count),
  both per-tile PE transposes, their PSUM->SBUF copies, and the per-tile
  bias matmul.
- dst-part contribution via host-precomputed onehot matmuls against a local
  per-block V table (V[n] = h[n] @ Wf_dst, no gather, no collective);
  onehots are streamed from DRAM per layer (too big for SBUF in f32).
- Scatter aggregation via host-precomputed onehots (no per-tile DVE
  is_equal).
- sigmoid*softplus computed slab-wide (4 tiles at a time) with 4 ACT passes
  (fused 2-wide Exp, Ln, Ln, Exp) + 7 DVE passes incl. fused
  scalar_tensor_tensor. f pre-acts are negated via negated Wf/bf so the
  sigmoid needs no reciprocal: sig = exp(-ln(1+e^-f)).
- The whole conv value path MUST be f32: the network amplifies per-layer
  relative error by ~1e5 (sigmoid gates flip), so bf16/f32r/fp16 anywhere
  in h/U/V/msg blows past the 2e-2 gate (verified empirically).
- Pre-act clamps to +-30 before Exp are mandatory: the Exp LUT returns
  garbage/NaN for far-out-of-range arguments (verified on HW).
"""
import sys

sys.path.insert(0, "/opt/trn_rl_repo")

import numpy as np
import ml_dtypes

import concourse.bacc as bacc
import concourse.bass as bass
import concourse.mybir as mybir
import concourse.tile as tile
from concourse.bass_utils import run_bass_kernel_spmd
from concourse.masks import make_identity

F32 = mybir.dt.float32
BF16 = mybir.dt.bfloat16
I16 = mybir.dt.int16
F32R = mybir.dt.float32r
NPBF = ml_dtypes.bfloat16

# Pin every ACT op to the one LUT set containing all functions we use
# (Exp, Ln, Relu, Copy, Identity). Without this, the table chooser can
# alternate tables, inserting ~1.3us ACT_TABLE_LOADs.
_orig_get_act_tables = bacc.get_activation_tables


def _pinned_act_tables(module_arch):
    tabs = dict(_orig_get_act_tables(module_arch))
    keep = "natural_log_exp_and_others"
    ours = {
        mybir.ActivationFunctionType.Exp,
        mybir.ActivationFunctionType.Ln,
        mybir.ActivationFunctionType.Relu,
        mybir.ActivationFunctionType.Copy,
        mybir.ActivationFunctionType.Identity,
    }
    out = {}
    for name, fns in tabs.items():
        out[name] = set(fns) if name == keep else (set(fns) - ours)
    return out


bacc.get_activation_tables = _pinned_act_tables

NCORES = 8
H = 128
NGRAPH = 256
EPS = 1e-5
NLAYERS = 10
GCH = 1024       # edges per bulk-gather chunk (>1024 hangs the SWDGE gather ucode)
SLAB = 4         # tiles per elementwise slab (SLAB*256 f32 = 2 PSUM banks)
CLAMP = 30.0     # pre-act clamp before Exp
AGB = 3          # node blocks per chunked U AllGather (overlaps collective w/ compute)


# ----------------------------------------------------------------------------
# Host-side preprocessing
# ----------------------------------------------------------------------------

def _prepare(inputs, n_layers=NLAYERS):
    x = np.asarray(inputs["x"], np.float32)              # [N, 10]
    ea = np.asarray(inputs["edge_attr"], np.float32)     # [E, 3]
    ei = np.asarray(inputs["edge_index"]).astype(np.int64)  # [2, E]
    batch = np.asarray(inputs["batch"]).astype(np.int64)    # [N] sorted
    N, E = x.shape[0], ea.shape[0]

    # graph -> node range (batch sorted)
    g_start = np.searchsorted(batch, np.arange(NGRAPH), side="left")
    g_end = np.searchsorted(batch, np.arange(NGRAPH), side="right")

    src, dst = ei[0], ei[1]
    e_graph = batch[dst]
    e_per_graph = np.bincount(e_graph, minlength=NGRAPH)

    # contiguous graph partition balanced by edge count
    cum = np.cumsum(e_per_graph)
    total = cum[-1]
    cuts = [0]
    for k in range(1, NCORES):
        cuts.append(int(np.searchsorted(cum, total * k / NCORES)))
    cuts.append(NGRAPH)
    g_lo = np.array(cuts[:-1])
    g_hi = np.array(cuts[1:])

    n_lo = np.array([g_start[g_lo[k]] if g_lo[k] < NGRAPH else N for k in range(NCORES)])
    n_hi = np.array([g_end[g_hi[k] - 1] if g_hi[k] > g_lo[k] else n_lo[k] for k in range(NCORES)])
    npc = n_hi - n_lo
    NB = int(np.ceil(npc.max() / 128))
    # AllGather chunk sizes (in 128-node blocks), in block-processing order:
    # big chunks early, tiny chunks last. Blocks finish bunched at the layer
    # end, so the last chunks' collectives otherwise serialize (~30us each)
    # past the layer boundary; 1-block tail chunks keep the exposed tail to
    # one small collective.
    if NB >= 8:
        head = NB - 4
        CHUNKS = []
        while head > 0:
            s = min(5, head)
            CHUNKS.append(s)
            head -= s
        CHUNKS += [2, 1, 1]
    else:
        CHUNKS = [NB]
    NCHUNK = len(CHUNKS)
    chunk_first = np.cumsum([0] + CHUNKS[:-1])          # first block of chunk
    chunk_of_block = np.repeat(np.arange(NCHUNK), CHUNKS)
    table_base = np.cumsum([0] + [NCORES * s * 128 for s in CHUNKS])  # row base
    NPC_PAD = NB * 128
    NPAD_G = int(table_base[-1])
    assert NPAD_G < 32768

    # Renumber nodes within each core so edge counts per 128-node block are
    # balanced (LPT binning by in-degree) — minimizes tile padding (T).
    perms = []
    core_of_node = np.zeros(N, np.int64)
    local_of_node = np.zeros(N, np.int64)
    for k in range(NCORES):
        sl = slice(n_lo[k], n_hi[k])
        core_of_node[sl] = k
        nk = int(npc[k])
        mask = (dst >= n_lo[k]) & (dst < n_hi[k])
        dl0 = dst[np.nonzero(mask)[0]] - n_lo[k]
        deg = np.bincount(dl0, minlength=max(nk, 1))
        order = np.argsort(-deg[:nk], kind="stable")
        perm = np.zeros(max(nk, 1), np.int64)
        bin_sum = np.zeros(NB, np.float64)
        bin_cnt = np.zeros(NB, np.int64)
        for nloc in order:
            cand = np.nonzero(bin_cnt < 128)[0]
            b = cand[np.argmin(bin_sum[cand])]
            perm[nloc] = b * 128 + bin_cnt[b]
            bin_cnt[b] += 1
            bin_sum[b] += deg[nloc]
        perms.append(perm)
        local_of_node[sl] = perm[:nk]
    # Global table row id under the chunked-AllGather layout:
    # U_tabs rows = [chunk][core][node-within-chunk]
    _blk = local_of_node // 128
    _chunk = chunk_of_block[_blk]
    _chunk_nodes = np.array(CHUNKS)[_chunk] * 128
    _within = local_of_node - chunk_first[_chunk] * 128
    gid_of_node = table_base[_chunk] + core_of_node * _chunk_nodes + _within

    # per-core edge lists grouped by dst block
    per_core_edges = []
    blk_counts = np.zeros((NCORES, NB), np.int64)
    for k in range(NCORES):
        mask = (dst >= n_lo[k]) & (dst < n_hi[k])
        eidx = np.nonzero(mask)[0]
        dl = perms[k][dst[eidx] - n_lo[k]]
        order = np.argsort(dl, kind="stable")
        eidx = eidx[order]
        dl = dl[order]
        blocks = dl // 128
        per_blk = [eidx[blocks == b] for b in range(NB)]
        per_core_edges.append(per_blk)
        for b in range(NB):
            blk_counts[k, b] = len(per_blk[b])

    TPB = np.maximum(1, np.ceil(blk_counts.max(axis=0) / 128).astype(np.int64))  # [NB]
    T = int(TPB.sum())
    EPC_PAD = T * 128

    G_MAX = int((g_hi - g_lo).max())
    n_per_graph = g_end - g_start
    assert n_per_graph.max() <= 128, "slot maxpool assumes <=128 nodes/graph"

    cfg = dict(NB=NB, NPC_PAD=NPC_PAD, NPAD_G=NPAD_G, T=T, EPC_PAD=EPC_PAD,
               TPB=tuple(int(t) for t in TPB), G_MAX=G_MAX, n_layers=n_layers,
               CHUNKS=tuple(int(s) for s in CHUNKS))

    def wrap16(idx):
        # [128, len/16] int16, replicated-wrap layout
        n = len(idx)
        assert n % 16 == 0
        w = np.zeros((16, n // 16), np.int16)
        w[np.arange(n) % 16, np.arange(n) // 16] = idx.astype(np.int16)
        return np.tile(w, (8, 1))

    # ---- shared (replicated) tensors ----
    wnode = np.zeros((11, H), np.float32)
    wnode[:10] = np.asarray(inputs["W_node"], np.float32)
    wnode[10] = np.asarray(inputs["b_node"], np.float32)

    wedge = np.zeros((4, H), np.float32)
    wedge[:3] = np.asarray(inputs["W_edge"], np.float32)
    wedge[3] = np.asarray(inputs["b_edge"], np.float32)

    # wfs: [128, nL*768] bf16; per layer i: [dst 256 | src 256 | e 256],
    # each 256 = [Wf part (NEGATED) | Ws part]. bfs: [1, nL*256] (f NEGATED).
    Wf = np.asarray(inputs["Wf"], np.float32)   # [10, 384, 128]
    Ws = np.asarray(inputs["Ws"], np.float32)
    bf = np.asarray(inputs["bf"], np.float32)   # [10, 128]
    bs = np.asarray(inputs["bs"], np.float32)
    wfs = np.zeros((128, n_layers * 768), np.float32)
    bfs = np.zeros((1, n_layers * 256), np.float32)
    for i in range(n_layers):
        for c in range(3):  # 0=dst(x_i) 1=src(x_j) 2=e
            col = i * 768 + c * 256
            wfs[:, col:col + 128] = -Wf[i, c * 128:(c + 1) * 128, :]
            wfs[:, col + 128:col + 256] = Ws[i, c * 128:(c + 1) * 128, :]
        bfs[0, i * 256:i * 256 + 128] = -bf[i]
        bfs[0, i * 256 + 128:(i + 1) * 256] = bs[i]

    bn_g = np.asarray(inputs["bn_g"], np.float64)
    bn_b = np.asarray(inputs["bn_b"], np.float64)
    bn_m = np.asarray(inputs["bn_m"], np.float64)
    bn_v = np.asarray(inputs["bn_v"], np.float64)
    scale = (bn_g / np.sqrt(bn_v + EPS)).astype(np.float32)   # [10, 128]
    shift = (bn_b - bn_m * (bn_g / np.sqrt(bn_v + EPS))).astype(np.float32)
    # feature-major: one column per layer, feature on the partition axis
    # (consumed as per-partition scale/bias APs by the Scalar engine)
    bns = scale[:n_layers].T.copy().astype(np.float32)   # [128, n_layers]
    bnb = shift[:n_layers].T.copy().astype(np.float32)

    iota = np.tile(np.arange(128, dtype=np.float32)[None, :], (128, 1))

    # heads
    W1 = np.asarray(inputs["W1"], np.float64)
    sc1 = (np.asarray(inputs["bn1_g"], np.float64) / np.sqrt(np.asarray(inputs["bn1_v"], np.float64) + EPS))
    sh1 = (np.asarray(inputs["b1"], np.float64) - np.asarray(inputs["bn1_m"], np.float64)) * sc1 + np.asarray(inputs["bn1_b"], np.float64)
    W2 = np.asarray(inputs["W2"], np.float64)
    sc2 = (np.asarray(inputs["bn2_g"], np.float64) / np.sqrt(np.asarray(inputs["bn2_v"], np.float64) + EPS))
    sh2 = (np.asarray(inputs["b2"], np.float64) - np.asarray(inputs["bn2_m"], np.float64)) * sc2 + np.asarray(inputs["bn2_b"], np.float64)
    W3 = np.asarray(inputs["W3"], np.float32)   # [128, 64]
    b3 = np.asarray(inputs["b3"], np.float32)   # [64]
    W4 = np.concatenate([np.asarray(inputs[n], np.float32) for n in ("Wv", "W_en", "Wd", "Wh")], axis=1)  # [64, 4]
    b4 = np.concatenate([np.asarray(inputs[n], np.float32) for n in ("bv", "b_en", "bd", "bh")])  # [4]

    w1p = np.zeros((128, 3 * 256), np.float32)
    for c in range(3):
        w1p[:, c * 256:(c + 1) * 256] = W1[c * 128:(c + 1) * 128, :]
    w2p = np.zeros((128, 2 * 128), np.float32)
    for c in range(2):
        w2p[:, c * 128:(c + 1) * 128] = W2[c * 128:(c + 1) * 128, :]
    w3p = W3.astype(np.float32)
    w4p = np.zeros((64, 4), np.float32)
    w4p[:, :] = W4

    hcol = np.zeros((128, 8), np.float32)
    hcol[:, 0] = sc1[:128]
    hcol[:, 1] = sc1[128:]
    hcol[:, 2] = sh1[:128]
    hcol[:, 3] = sh1[128:]
    hcol[:, 4] = sc2
    hcol[:, 5] = sh2
    hcol[:64, 6] = b3
    hcol[:4, 7] = b4

    shared = dict(wnode=wnode, wedge=wedge, wfs=wfs, bfs=bfs,
                  bns=bns, bnb=bnb, iota=iota, w1p=w1p, w2p=w2p, w3p=w3p,
                  w4p=w4p, hcol=hcol)

    # ---- per-core tensors ----
    in_maps = []
    meta = []
    for k in range(NCORES):
        xT_own = np.zeros((11, NPC_PAD), np.float32)
        xT_own[:10, perms[k][:npc[k]]] = x[n_lo[k]:n_hi[k]].T
        xT_own[10] = 1.0

        eaT = np.zeros((4, EPC_PAD), np.float32)
        eaT[3] = 1.0
        src_ids = np.zeros(EPC_PAD, np.int64)
        dst_rel = np.full(EPC_PAD, -1, np.int64)
        pos = 0
        for b in range(NB):
            eidx = per_core_edges[k][b]
            ne = len(eidx)
            cap = int(TPB[b]) * 128
            assert ne <= cap
            eaT[:3, pos:pos + ne] = ea[eidx].T
            src_ids[pos:pos + ne] = gid_of_node[src[eidx]]
            dst_rel[pos:pos + ne] = perms[k][dst[eidx] - n_lo[k]] - b * 128
            pos += cap
        assert pos == EPC_PAD

        srcg = wrap16(src_ids)

        # onehots: oh1[p=edge-in-tile, t*128 + node] for scatter lhsT;
        #          oh2[p=node, t*128 + edge-in-tile] for dst-part lhsT
        oh1 = np.zeros((128, T * 128), np.float32)
        oh2 = np.zeros((128, T * 128), np.float32)
        tt = np.arange(EPC_PAD) // 128
        pp = np.arange(EPC_PAD) % 128
        valid = dst_rel >= 0
        oh1[pp[valid], tt[valid] * 128 + dst_rel[valid]] = 1.0
        oh2[dst_rel[valid], tt[valid] * 128 + pp[valid]] = 1.0

        invp = np.full(NPC_PAD, -1, np.int64)
        invp[perms[k][:npc[k]]] = np.arange(npc[k])
        grel = np.full((128, NB), -1.0, np.float32)
        for b in range(NB):
            for p in range(128):
                orig = invp[b * 128 + p]
                if orig >= 0:
                    grel[p, b] = float(batch[n_lo[k] + orig] - g_lo[k])

        Gk = int(g_hi[k] - g_lo[k])
        invcnt = np.ones((128, 1), np.float32)
        slot_ids = np.zeros(G_MAX * 128, np.int64)
        for gl in range(G_MAX):
            g = g_lo[k] + gl
            if gl < Gk:
                nodes = np.arange(g_start[g], g_end[g])
                cnt = len(nodes)
                invcnt[gl, 0] = 1.0 / max(cnt, 1)
                sl = perms[k][nodes - n_lo[k]]
                slots = np.resize(sl, 128) if cnt > 0 else np.zeros(128, np.int64)
            else:
                slots = np.zeros(128, np.int64)
            slot_ids[gl * 128:(gl + 1) * 128] = slots
        slotg = wrap16(slot_ids)

        m = dict(shared)
        m.update(xT_own=xT_own, eaT=eaT, srcg=srcg, oh1=oh1, oh2=oh2,
                 grel=grel, invcnt=invcnt, slotg=slotg)
        in_maps.append(m)
        meta.append(dict(g_lo=int(g_lo[k]), g_hi=int(g_hi[k])))

    return in_maps, cfg, meta


# ----------------------------------------------------------------------------
# Bass program
# ----------------------------------------------------------------------------

def _build(cfg, debug_dump=False):
    NB = cfg["NB"]
    NPC_PAD = cfg["NPC_PAD"]
    NPAD_G = cfg["NPAD_G"]
    T = cfg["T"]
    EPC_PAD = cfg["EPC_PAD"]
    TPB = cfg["TPB"]
    G_MAX = cfg["G_MAX"]
    n_layers = cfg["n_layers"]
    CHUNKS = cfg["CHUNKS"]
    NCHUNK = len(CHUNKS)
    chunk_first = [0]
    for s in CHUNKS[:-1]:
        chunk_first.append(chunk_first[-1] + s)
    chunk_of_block = []
    for c, s in enumerate(CHUNKS):
        chunk_of_block += [c] * s
    table_base = [0]
    for s in CHUNKS:
        table_base.append(table_base[-1] + NCORES * s * 128)

    nc = bacc.Bacc("TRN2", debug=False, num_devices=NCORES)

    d_xT_own = nc.dram_tensor("xT_own", [11, NPC_PAD], F32, kind="ExternalInput")
    d_eaT = nc.dram_tensor("eaT", [4, EPC_PAD], F32, kind="ExternalInput")
    d_srcg = nc.dram_tensor("srcg", [128, EPC_PAD // 16], I16, kind="ExternalInput")
    d_oh1 = nc.dram_tensor("oh1", [128, T * 128], F32, kind="ExternalInput")
    d_oh2 = nc.dram_tensor("oh2", [128, T * 128], F32, kind="ExternalInput")
    d_grel = nc.dram_tensor("grel", [128, NB], F32, kind="ExternalInput")
    d_invcnt = nc.dram_tensor("invcnt", [128, 1], F32, kind="ExternalInput")
    d_slotg = nc.dram_tensor("slotg", [128, G_MAX * 128 // 16], I16, kind="ExternalInput")
    d_wnode = nc.dram_tensor("wnode", [11, H], F32, kind="ExternalInput")
    d_wedge = nc.dram_tensor("wedge", [4, H], F32, kind="ExternalInput")
    d_wfs = nc.dram_tensor("wfs", [128, n_layers * 768], F32, kind="ExternalInput")
    d_bfs = nc.dram_tensor("bfs", [1, n_layers * 256], F32, kind="ExternalInput")
    d_bns = nc.dram_tensor("bns", [128, n_layers], F32, kind="ExternalInput")
    d_bnb = nc.dram_tensor("bnb", [128, n_layers], F32, kind="ExternalInput")
    d_iota = nc.dram_tensor("iota", [128, 128], F32, kind="ExternalInput")
    d_w1p = nc.dram_tensor("w1p", [128, 3 * 256], F32, kind="ExternalInput")
    d_w2p = nc.dram_tensor("w2p", [128, 2 * 128], F32, kind="ExternalInput")
    d_w3p = nc.dram_tensor("w3p", [128, 64], F32, kind="ExternalInput")
    d_w4p = nc.dram_tensor("w4p", [64, 4], F32, kind="ExternalInput")
    d_hcol = nc.dram_tensor("hcol", [128, 8], F32, kind="ExternalInput")

    d_out4 = nc.dram_tensor("out4", [4, G_MAX], F32, kind="ExternalOutput")
    if debug_dump:
        d_hdump = nc.dram_tensor("hdump", [NPC_PAD, H], F32, kind="ExternalOutput")

    AF = mybir.ActivationFunctionType
    ALU = mybir.AluOpType

    with tile.TileContext(nc) as tc, nc.allow_low_precision(reason="bf16 conv path; PSUM accumulation stays f32"):
        import contextlib
        ctx = contextlib.ExitStack()
        with ctx:
            cpool = ctx.enter_context(tc.tile_pool(name="const", bufs=1))
            dram = ctx.enter_context(tc.tile_pool(name="dram", bufs=1, space="DRAM"))
            work = ctx.enter_context(tc.tile_pool(name="work", bufs=2))
            gbuf = ctx.enter_context(tc.tile_pool(name="gbuf", bufs=2))
            psum_p = ctx.enter_context(tc.tile_pool(name="psum_p", bufs=2, space="PSUM"))
            psum_a = ctx.enter_context(tc.tile_pool(name="psum_a", bufs=2, space="PSUM"))
            psum_t = ctx.enter_context(tc.tile_pool(name="psum_t", bufs=1, space="PSUM"))

            # DRAM state
            eT_d = dram.tile([128, EPC_PAD], F32, name="eT_d")
            # Per-chunk U staging. Separate tensors so each chunk's AllGather
            # depends only on its own blocks' U writes (collective overlaps
            # remaining compute).
            U_own_c = [dram.tile([CHUNKS[c] * 128, 256], F32, name=f"U_own{c}")
                       for c in range(NCHUNK)]
            # NOTE: plain (Local) DRAM, not addr_space="Shared" — the CoreSim
            # race detector enforces one writer instruction per Shared
            # tensor, which forbids the per-chunk AllGathers. Local-output
            # collectives bounce through per-collective internal buffers.
            U_tabs = []
            for i in range(n_layers):
                U_tabs.append(dram.tile([NPAD_G, 256], F32, name=f"U_tab{i}"))
            hin_slice = dram.tile([NPC_PAD, H], F32, name="hin_slice")

            # constants in SBUF
            ident = cpool.tile([128, 128], F32)
            make_identity(nc, ident[:])
            ones16 = cpool.tile([1, 128], F32)
            nc.vector.memset(ones16[:], 1.0)
            c_wnode = cpool.tile([11, H], F32)
            nc.sync.dma_start(out=c_wnode[:], in_=d_wnode[:])
            c_wedge = cpool.tile([4, H], F32)
            nc.sync.dma_start(out=c_wedge[:], in_=d_wedge[:])
            c_wfs = cpool.tile([128, n_layers * 768], F32)
            nc.sync.dma_start(out=c_wfs[:], in_=d_wfs[:])
            c_bfs = cpool.tile([1, n_layers * 256], F32)
            nc.sync.dma_start(out=c_bfs[:], in_=d_bfs[:])
            c_bns = cpool.tile([128, n_layers], F32)
            nc.sync.dma_start(out=c_bns[:], in_=d_bns[:])
            c_bnb = cpool.tile([128, n_layers], F32)
            nc.sync.dma_start(out=c_bnb[:], in_=d_bnb[:])
            c_iota = cpool.tile([128, 128], F32)
            nc.sync.dma_start(out=c_iota[:], in_=d_iota[:])
            c_srcg = cpool.tile([128, EPC_PAD // 16], I16)
            nc.sync.dma_start(out=c_srcg[:], in_=d_srcg[:])
            c_grel = cpool.tile([128, NB], F32)
            nc.sync.dma_start(out=c_grel[:], in_=d_grel[:])
            c_invcnt = cpool.tile([128, 1], F32)
            nc.sync.dma_start(out=c_invcnt[:], in_=d_invcnt[:])
            c_slotg = cpool.tile([128, G_MAX * 128 // 16], I16)
            nc.sync.dma_start(out=c_slotg[:], in_=d_slotg[:])
            c_w1p = cpool.tile([128, 3 * 256], F32)
            nc.sync.dma_start(out=c_w1p[:], in_=d_w1p[:])
            c_w2p = cpool.tile([128, 2 * 128], F32)
            nc.sync.dma_start(out=c_w2p[:], in_=d_w2p[:])
            c_w3p = cpool.tile([128, 64], F32)
            nc.sync.dma_start(out=c_w3p[:], in_=d_w3p[:])
            c_w4p = cpool.tile([64, 4], F32)
            nc.sync.dma_start(out=c_w4p[:], in_=d_w4p[:])
            c_hcol = cpool.tile([128, 8], F32)
            nc.sync.dma_start(out=c_hcol[:], in_=d_hcol[:])

            # persistent SBUF state.
            # h_own is FEATURE-major: [feat(p), block*128 + node]. This makes
            # h blocks directly usable as matmul lhsT for the U/V tables (no
            # per-block PE transpose + PSUM copy), and turns the BN
            # scale/shift into per-partition ACT scale/bias.
            h_own = cpool.tile([128, NPC_PAD], F32, name="h_own")
            # node-major copy of the final h (for pooling), written once.
            h_nm_all = cpool.tile([128, NPC_PAD], F32, name="h_nm_all")
            V_all = cpool.tile([128, NB * 256], F32, name="V_all")

            def emit_block_post(i_next, b):
                """After h_own block b is final: compute V (dst) and U (src)
                tables for layer i_next; DMA U slice to its chunk's staging
                buffer; when the chunk is complete, AllGather it into
                U_tabs[i_next] (overlapping remaining blocks' compute)."""
                bs_ = slice(b * 128, (b + 1) * 128)
                # V (dst part) | U (src part) in one N=512 matmul; bias rides
                # a K=1 accumulate into the U half. h_own is feature-major so
                # the h block IS the lhsT — no transpose needed.
                pall = psum_t.tile([128, 512], F32, tag="uv", bufs=2)
                nc.tensor.matmul(out=pall[:], lhsT=h_own[:, bs_],
                                 rhs=c_wfs[:, i_next * 768:i_next * 768 + 512],
                                 start=True, stop=False)
                nc.tensor.matmul(out=pall[:, 256:512], lhsT=ones16[:],
                                 rhs=c_bfs[:, i_next * 256:(i_next + 1) * 256],
                                 start=False, stop=True)
                nc.vector.tensor_copy(out=V_all[:, b * 256:(b + 1) * 256],
                                      in_=pall[:, 0:256])
                u32 = work.tile([128, 256], F32, tag="u32")
                nc.scalar.copy(out=u32[:], in_=pall[:, 256:512])
                c = chunk_of_block[b]
                boff = b - chunk_first[c]
                nc.sync.dma_start(
                    out=U_own_c[c][boff * 128:(boff + 1) * 128, :],
                    in_=u32[:])
                if b - chunk_first[c] == CHUNKS[c] - 1:
                    nc.gpsimd.collective_compute(
                        "AllGather", ALU.bypass,
                        replica_groups=[list(range(NCORES))],
                        ins=[U_own_c[c].opt()],
                        outs=[U_tabs[i_next][table_base[c]:table_base[c + 1], :].opt()])

            with tc.tile_pool(name="enc", bufs=1) as enc:
                # ---------------- encoder: own nodes (first, so the layer-0
                # U AllGather chunks launch ASAP and overlap the edge
                # encoder) ----------------
                for b in range(NB):
                    xo_sb = enc.tile([11, 128], F32, tag="xo_sb", bufs=2)
                    nc.sync.dma_start(out=xo_sb[:], in_=d_xT_own[:, b * 128:(b + 1) * 128])
                    ph = psum_a.tile([128, 128], F32, tag="agg")
                    # lhsT=W, rhs=x -> out [feat, node] (feature-major h)
                    nc.tensor.matmul(out=ph[:], lhsT=c_wnode[:],
                                     rhs=xo_sb[:], start=True, stop=True)
                    nc.scalar.activation(h_own[:, b * 128:(b + 1) * 128], ph[:], AF.Relu)
                    emit_block_post(0, b)

                # ---------------- encoder: edges ----------------
                # Triple-buffered: with bufs=1 the DMA->MM->relu->DMA chain
                # fully serialized (~3.5us/chunk, ~300us startup stall before
                # layer 0's first slab).
                for c in range(0, EPC_PAD, 256):
                    w = min(256, EPC_PAD - c)
                    ea_sb = enc.tile([4, 256], F32, tag="ea_sb", bufs=3)
                    nc.sync.dma_start(out=ea_sb[:, :w], in_=d_eaT[:, c:c + w])
                    pe = psum_p.tile([128, SLAB, 256], F32, tag="P")
                    nc.tensor.matmul(out=pe[:, 0, :w],
                                     lhsT=c_wedge[:], rhs=ea_sb[:, :w],
                                     start=True, stop=True)
                    et_sb = enc.tile([128, 256], F32, tag="et_sb", bufs=2)
                    nc.scalar.activation(et_sb[:, :w], pe[:, 0, :w], AF.Relu)
                    nc.sync.dma_start(out=eT_d[:, c:c + w], in_=et_sb[:, :w])

            # tile index -> block
            tile_block = []
            for b in range(NB):
                tile_block += [b] * TPB[b]
            first_tile_of_block = {}
            last_tile_of_block = {}
            for t, b in enumerate(tile_block):
                if b not in first_tile_of_block:
                    first_tile_of_block[b] = t
                last_tile_of_block[b] = t

            n_gch = (EPC_PAD + GCH - 1) // GCH
            n_slab = (T + SLAB - 1) // SLAB

            def block_finish(i, b, agg):
                # agg is [feat, node] (scatter matmul emits feature-major);
                # BN scale/shift are per-feature = per-partition, fused into
                # the ACT Relu as scale/bias.
                bs_ = slice(b * 128, (b + 1) * 128)
                t0 = work.tile([128, 128], F32, tag="t0")
                nc.vector.tensor_tensor(out=t0[:], in0=agg[:], in1=h_own[:, bs_],
                                        op=ALU.add)
                if i % 2 == 1:
                    t3 = work.tile([128, 128], F32, tag="t3")
                    nc.scalar.activation(t3[:], t0[:], AF.Relu,
                                         bias=c_bnb[:, i:i + 1],
                                         scale=c_bns[:, i:i + 1])
                    nc.vector.tensor_tensor(out=h_own[:, bs_], in0=t3[:],
                                            in1=h_own[:, bs_], op=ALU.add)
                else:
                    nc.scalar.activation(h_own[:, bs_], t0[:], AF.Relu,
                                         bias=c_bnb[:, i:i + 1],
                                         scale=c_bns[:, i:i + 1])
                if i < n_layers - 1:
                    emit_block_post(i + 1, b)
                else:
                    # node-major copy for pooling + the maxpool slot gather.
                    # (psum_a "agg" tag: only live at the last layer, so it
                    # shares banks with the scatter aggregators instead of
                    # costing psum_t a dedicated bank.)
                    tp = psum_a.tile([128, 128], F32, tag="agg")
                    nc.tensor.transpose(out=tp[:], in_=h_own[:, bs_],
                                        identity=ident[:])
                    nc.scalar.copy(out=h_nm_all[:, bs_], in_=tp[:])
                    nc.sync.dma_start(out=hin_slice[b * 128:(b + 1) * 128, :],
                                      in_=h_nm_all[:, bs_])

            # ---------------- conv layers ----------------
            for i in range(n_layers):
                u_t = []
                eTb_t = []
                oh1_t = []
                oh2_t = []
                for c in range(n_gch):
                    lo = c * GCH
                    hi = min(EPC_PAD, lo + GCH)
                    w = hi - lo
                    ug = gbuf.tile([128, GCH // 128, 256], F32, tag="ug", bufs=3)
                    nc.gpsimd.dma_gather(
                        out_ap=ug[:, :w // 128, :], in_ap=U_tabs[i][:],
                        idxs_ap=c_srcg[:, lo // 16:hi // 16],
                        num_idxs=w, num_idxs_reg=w, elem_size=256)
                    eTb = gbuf.tile([128, GCH], F32, tag="eTb", bufs=2)
                    nc.sync.dma_start(out=eTb[:, :w], in_=eT_d[:, lo:hi])
                    oh1b = gbuf.tile([128, GCH], F32, tag="oh1b", bufs=2)
                    nc.sync.dma_start(out=oh1b[:, :w], in_=d_oh1[:, lo:hi])
                    oh2b = gbuf.tile([128, GCH], F32, tag="oh2b", bufs=2)
                    nc.sync.dma_start(out=oh2b[:, :w], in_=d_oh2[:, lo:hi])
                    u_t.append(ug)
                    eTb_t.append(eTb)
                    oh1_t.append(oh1b)
                    oh2_t.append(oh2b)

                wcol = i * 768
                pend = []  # (tiles, MSG tile) awaiting scatter

                def emit_scatter(tiles, MSG, aggs, i=i):
                    for j, t in enumerate(tiles):
                        b = tile_block[t]
                        ch, off = t * 128 // GCH, (t * 128 % GCH) // 128
                        if t == first_tile_of_block[b]:
                            aggs[b] = psum_a.tile([128, 128], F32, tag="agg",
                                                  name="agg")
                        # lhsT=MSG, rhs=onehot -> agg [feat, node]: same
                        # products/accumulation order as the node-major form
                        # (contraction over the same 128 edges), but the
                        # output lands feature-major for free.
                        nc.tensor.matmul(
                            out=aggs[b][:],
                            lhsT=MSG[:, j * 128:(j + 1) * 128],
                            rhs=oh1_t[ch][:, off * 128:(off + 1) * 128],
                            start=(t == first_tile_of_block[b]),
                            stop=(t == last_tile_of_block[b]))
                        if t == last_tile_of_block[b]:
                            block_finish(i, b, aggs.pop(b))

                aggs = {}
                for s in range(n_slab):
                    t0_ = s * SLAB
                    tiles = list(range(t0_, min(T, t0_ + SLAB)))
                    nj = len(tiles)
                    P = psum_p.tile([128, SLAB, 256], F32, tag="P")
                    for j, t in enumerate(tiles):
                        b = tile_block[t]
                        ch, off = t * 128 // GCH, (t * 128 % GCH) // 128
                        nc.tensor.matmul(out=P[:, j, :],
                                         lhsT=oh2_t[ch][:, off * 128:(off + 1) * 128],
                                         rhs=V_all[:, b * 256:(b + 1) * 256],
                                         start=True, stop=False)
                        nc.tensor.matmul(out=P[:, j, :],
                                         lhsT=eTb_t[ch][:, off * 128:(off + 1) * 128],
                                         rhs=c_wfs[:, wcol + 512:wcol + 768],
                                         start=False, stop=True)
                    ch0 = t0_ * 128 // GCH
                    off0 = (t0_ * 128 % GCH) // 128
                    w1 = nj * 128

                    # FS de-interleaved: FS[:,0,:]=-f, FS[:,1,:]=s — two half
                    # adds pay the strided PSUM/u read once so every later
                    # elementwise op runs on contiguous SBUF. f32: exp args
                    # must not be bf16-rounded — values reach +-40k.
                    # Fresh-output tiles for the tensor_scalar ops; fused
                    # 2-wide Exp (instruction count beats per-op density).
                    FS = work.tile([128, 2, SLAB * 128], F32, tag="FS")
                    nc.vector.tensor_tensor(
                        out=FS[:, 0, :w1], in0=P[:, :nj, 0:128],
                        in1=u_t[ch0][:, off0:off0 + nj, 0:128], op=ALU.add)
                    nc.vector.tensor_tensor(
                        out=FS[:, 1, :w1], in0=P[:, :nj, 128:256],
                        in1=u_t[ch0][:, off0:off0 + nj, 128:256], op=ALU.add)
                    # RS = max(s, 0) on the (less busy) Scalar engine
                    RS = work.tile([128, SLAB * 128], F32, tag="RS")
                    nc.scalar.activation(RS[:, :w1], FS[:, 1, :w1], AF.Relu)
                    # G[:,0] = clamp(-f, +-30); G[:,1] = max(-|s|, -30)
                    # (Exp LUT yields NaN/garbage for out-of-range args)
                    G = work.tile([128, 2, SLAB * 128], F32, tag="G")
                    nc.vector.tensor_scalar(
                        out=G[:, 0, :w1], in0=FS[:, 0, :w1],
                        scalar1=-CLAMP, scalar2=CLAMP, op0=ALU.max, op1=ALU.min)
                    T1 = work.tile([128, SLAB * 128], F32, tag="T1", bufs=1)
                    nc.vector.scalar_tensor_tensor(
                        out=T1[:, :w1], in0=FS[:, 1, :w1], scalar=0.0,
                        in1=RS[:, :w1], op0=ALU.min, op1=ALU.subtract)
                    nc.vector.tensor_scalar_max(out=G[:, 1, :w1],
                                                in0=T1[:, :w1], scalar1=-CLAMP)
                    # E = exp(G) (one fused 2-region pass); L = ln(1+E) split
                    # so the sigmoid path stays f32: L_f = ln(1+e^-f) = -ln(sig)
                    E = work.tile([128, 2, SLAB * 128], F32, tag="E", bufs=1)
                    nc.scalar.activation(E[:, :, :w1], G[:, :, :w1], AF.Exp)
                    LF = work.tile([128, SLAB * 128], F32, tag="LF", bufs=1)
                    nc.scalar.activation(LF[:, :w1], E[:, 0, :w1], AF.Ln, bias=1.0)
                    LS = work.tile([128, SLAB * 128], F32, tag="LS")
                    nc.scalar.activation(LS[:, :w1], E[:, 1, :w1], AF.Ln, bias=1.0)
                    SG = work.tile([128, SLAB * 128], F32, tag="SG")
                    nc.scalar.activation(SG[:, :w1], LF[:, :w1], AF.Exp, scale=-1.0)
                    SP = work.tile([128, SLAB * 128], F32, tag="SP")
                    nc.vector.tensor_tensor(out=SP[:, :w1], in0=RS[:, :w1],
                                            in1=LS[:, :w1], op=ALU.add)
                    MSG = work.tile([128, SLAB * 128], F32, tag="MSG")
                    nc.vector.tensor_tensor(out=MSG[:, :w1], in0=SP[:, :w1],
                                            in1=SG[:, :w1], op=ALU.mult)
                    pend.append((tiles, MSG))
                    if len(pend) > 1:
                        emit_scatter(*pend.pop(0), aggs)
                while pend:
                    emit_scatter(*pend.pop(0), aggs)
                # (U exchange for the next layer is issued per-chunk inside
                # emit_block_post, overlapping the rest of this layer.)

            if debug_dump:
                for b in range(NB):
                    nc.sync.dma_start(out=d_hdump[b * 128:(b + 1) * 128, :],
                                      in_=h_nm_all[:, b * 128:(b + 1) * 128])

            # ---------------- pooling ----------------
            ppool = psum_a.tile([128, 128], F32, tag="agg")
            for b in range(NB):
                ohg = work.tile([128, 128], F32, tag="ohg")
                nc.vector.tensor_tensor(
                    out=ohg[:], in0=c_grel[:, b:b + 1].to_broadcast([128, 128]),
                    in1=c_iota[:], op=ALU.is_equal)
                nc.tensor.matmul(out=ppool[:], lhsT=ohg[:],
                                 rhs=h_nm_all[:, b * 128:(b + 1) * 128],
                                 start=(b == 0), stop=(b == NB - 1))
            sum_nm = work.tile([128, 128], F32, tag="sum_nm")
            nc.vector.tensor_copy(out=sum_nm[:], in_=ppool[:])
            mean_nm = work.tile([128, 128], F32, tag="mean_nm")
            nc.scalar.activation(mean_nm[:], ppool[:], AF.Identity, scale=c_invcnt[:])

            gT = cpool.tile([128, 3 * G_MAX], F32, name="gT")
            pt = psum_a.tile([128, 128], F32, tag="agg")
            nc.tensor.transpose(out=pt[:], in_=mean_nm[:], identity=ident[:])
            nc.scalar.copy(out=gT[:, 0:G_MAX], in_=pt[:, 0:G_MAX])
            pt2 = psum_a.tile([128, 128], F32, tag="agg")
            nc.tensor.transpose(out=pt2[:], in_=sum_nm[:], identity=ident[:])
            nc.scalar.copy(out=gT[:, 2 * G_MAX:3 * G_MAX], in_=pt2[:, 0:G_MAX])

            # max pool via slot gather
            n_sch = (G_MAX * 128 + GCH - 1) // GCH
            gslot_t = []
            for c in range(n_sch):
                lo = c * GCH
                hi = min(G_MAX * 128, lo + GCH)
                w = hi - lo
                gslot = gbuf.tile([128, GCH // 128, H], F32, tag="gslot")
                nc.gpsimd.dma_gather(
                    out_ap=gslot[:, :w // 128, :], in_ap=hin_slice[:],
                    idxs_ap=c_slotg[:, lo // 16:hi // 16],
                    num_idxs=w, num_idxs_reg=w, elem_size=H)
                gslot_t.append(gslot)
            for g in range(G_MAX):
                ch, off = g * 128 // GCH, (g * 128 % GCH) // 128
                ptm = psum_a.tile([128, 128], F32, tag="agg")
                nc.tensor.transpose(out=ptm[:], in_=gslot_t[ch][:, off, :], identity=ident[:])
                nc.vector.reduce_max(out=gT[:, G_MAX + g:G_MAX + g + 1], in_=ptm[:],
                                     axis=mybir.AxisListType.X)

            # ---------------- heads ----------------
            p1a = psum_p.tile([128, SLAB, 256], F32, tag="P")
            p1b = psum_p.tile([128, SLAB, 256], F32, tag="P")
            for c in range(3):
                rhs = gT[:, c * G_MAX:(c + 1) * G_MAX]
                nc.tensor.matmul(out=p1a[:, 0, :G_MAX], lhsT=c_w1p[:, c * 256:c * 256 + 128],
                                 rhs=rhs, start=(c == 0), stop=(c == 2))
                nc.tensor.matmul(out=p1b[:, 0, :G_MAX], lhsT=c_w1p[:, c * 256 + 128:(c + 1) * 256],
                                 rhs=rhs, start=(c == 0), stop=(c == 2))
            g1a = work.tile([128, G_MAX], F32, tag="g1a")
            nc.scalar.activation(g1a[:], p1a[:, 0, :G_MAX], AF.Relu, bias=c_hcol[:, 2:3], scale=c_hcol[:, 0:1])
            g1b = work.tile([128, G_MAX], F32, tag="g1b")
            nc.scalar.activation(g1b[:], p1b[:, 0, :G_MAX], AF.Relu, bias=c_hcol[:, 3:4], scale=c_hcol[:, 1:2])

            p2 = psum_p.tile([128, SLAB, 256], F32, tag="P")
            nc.tensor.matmul(out=p2[:, 0, :G_MAX], lhsT=c_w2p[:, 0:128], rhs=g1a[:], start=True, stop=False)
            nc.tensor.matmul(out=p2[:, 0, :G_MAX], lhsT=c_w2p[:, 128:256], rhs=g1b[:], start=False, stop=True)
            g2 = work.tile([128, G_MAX], F32, tag="g2")
            nc.scalar.activation(g2[:], p2[:, 0, :G_MAX], AF.Relu, bias=c_hcol[:, 5:6], scale=c_hcol[:, 4:5])

            p3 = psum_p.tile([128, SLAB, 256], F32, tag="P")
            nc.tensor.matmul(out=p3[:64, 0, :G_MAX], lhsT=c_w3p[:], rhs=g2[:], start=True, stop=True)
            g3 = work.tile([64, G_MAX], F32, tag="g3")
            nc.scalar.activation(g3[:], p3[:64, 0, :G_MAX], AF.Relu, bias=c_hcol[:64, 6:7])

            p4 = psum_p.tile([128, SLAB, 256], F32, tag="P")
            nc.tensor.matmul(out=p4[:4, 0, :G_MAX], lhsT=c_w4p[:], rhs=g3[:], start=True, stop=True)
            o4 = work.tile([4, G_MAX], F32, tag="o4")
            nc.scalar.activation(o4[:], p4[:4, 0, :G_MAX], AF.Identity, bias=c_hcol[:4, 7:8])
            nc.sync.dma_start(out=d_out4[:], in_=o4[:])

    nc.compile()
    return nc


# ----------------------------------------------------------------------------
# Entry point
# ----------------------------------------------------------------------------

_CACHE = {}


def kernel(trace=False, n_layers=NLAYERS, debug_dump=False, **inputs):
    in_maps, cfg, meta = _prepare(inputs, n_layers=n_layers)
    key = (tuple(sorted(cfg.items())), debug_dump)
    if key not in _CACHE:
        _CACHE[key] = _build(cfg, debug_dump=debug_dump)
    nc = _CACHE[key]

    res = run_bass_kernel_spmd(nc, in_maps, core_ids=list(range(NCORES)), trace=trace)

    outs = [np.zeros((NGRAPH, 1), np.float32) for _ in range(4)]
    for k in range(NCORES):
        g_lo, g_hi = meta[k]["g_lo"], meta[k]["g_hi"]
        o4 = res.results[k]["out4"]   # [4, G_MAX]
        for j in range(4):
            outs[j][g_lo:g_hi, 0] = o4[j, :g_hi - g_lo]
    kernel._last_res = res
    if debug_dump:
        kernel._last_hdump = [res.results[k]["hdump"] for k in range(NCORES)]
        kernel._last_cfg = cfg
    return tuple(outs)



# revision 39
# speedup vs baseline: 1.0566x; 1.0566x over previous
"""Trainium2 Bass kernel for nn_BatteryGNN (CGConv message-passing GNN).

Self-contained: takes full inputs, shards graph-data-parallel across 8
NeuronCores, runs a single SPMD NEFF (10 CGConv layers + pooling + MLP heads),
gathers per-core head outputs on the host.

Design (vs original baseline):
- Per-edge work restructured around a per-layer "U table":
  U[n] = h[n] @ [Wf_src | Ws_src] + [bf | bs]  (256 wide, f32),
  computed per owned 128-node block, all-gathered, then ONE dma_gather per
  edge chunk fetches U[src] rows directly in [edge, 256] pre-act layout.
  This kills the baseline's dst gather (halves the SWDGE gather count),
  both per-tile PE transposes, their PSUM->SBUF copies, and the per-tile
  bias matmul.
- dst-part contribution via host-precomputed onehot matmuls against a local
  per-block V table (V[n] = h[n] @ Wf_dst, no gather, no collective);
  onehots are streamed from DRAM per layer (too big for SBUF in f32).
- Scatter aggregation via host-precomputed onehots (no per-tile DVE
  is_equal).
- sigmoid*softplus computed slab-wide (4 tiles at a time) with 4 ACT passes
  (fused 2-wide Exp, Ln, Ln, Exp) + 7 DVE passes incl. fused
  scalar_tensor_tensor. f pre-acts are negated via negated Wf/bf so the
  sigmoid needs no reciprocal: sig = exp(-ln(1+e^-f)).
- The whole conv value path MUST be f32: the network amplifies per-layer
  relative error by ~1e5 (sigmoid gates flip), so bf16/f32r/fp16 anywhere
  in h/U/V/msg blows past the 2e-2 gate (verified empirically).
- Pre-act clamps to +-30 before Exp are mandatory: the Exp LUT returns
  garbage/NaN for far-out-of-range arguments (verified on HW).
"""
import sys

sys.path.insert(0, "/opt/trn_rl_repo")

import numpy as np
import ml_dtypes

import concourse.bacc as bacc
import concourse.bass as bass
import concourse.mybir as mybir
import concourse.tile as tile
from concourse.bass_utils import run_bass_kernel_spmd
from concourse.masks import make_identity

F32 = mybir.dt.float32
BF16 = mybir.dt.bfloat16
I16 = mybir.dt.int16
F32R = mybir.dt.float32r
NPBF = ml_dtypes.bfloat16

# Pin every ACT op to the one LUT set containing all functions we use
# (Exp, Ln, Relu, Copy, Identity). Without this, the table chooser can
# alternate tables, inserting ~1.3us ACT_TABLE_LOADs.
_orig_get_act_tables = bacc.get_activation_tables


def _pinned_act_tables(module_arch):
    tabs = dict(_orig_get_act_tables(module_arch))
    keep = "natural_log_exp_and_others"
    ours = {
        mybir.ActivationFunctionType.Exp,
        mybir.ActivationFunctionType.Ln,
        mybir.ActivationFunctionType.Relu,
        mybir.ActivationFunctionType.Copy,
        mybir.ActivationFunctionType.Identity,
    }
    out = {}
    for name, fns in tabs.items():
        out[name] = set(fns) if name == keep else (set(fns) - ours)
    return out


bacc.get_activation_tables = _pinned_act_tables

NCORES = 8
H = 128
NGRAPH = 256
EPS = 1e-5
NLAYERS = 10
GCH = 1024       # edges per bulk-gather chunk (>1024 hangs the SWDGE gather ucode)
SLAB = 4         # tiles per elementwise slab (SLAB*256 f32 = 2 PSUM banks)
CLAMP = 30.0     # pre-act clamp before Exp
AGB = 3          # node blocks per chunked U AllGather (overlaps collective w/ compute)


# ----------------------------------------------------------------------------
# Host-side preprocessing
# ----------------------------------------------------------------------------

def _prepare(inputs, n_layers=NLAYERS):
    x = np.asarray(inputs["x"], np.float32)              # [N, 10]
    ea = np.asarray(inputs["edge_attr"], np.float32)     # [E, 3]
    ei = np.asarray(inputs["edge_index"]).astype(np.int64)  # [2, E]
    batch = np.asarray(inputs["batch"]).astype(np.int64)    # [N] sorted
    N, E = x.shape[0], ea.shape[0]

    # graph -> node range (batch sorted)
    g_start = np.searchsorted(batch, np.arange(NGRAPH), side="left")
    g_end = np.searchsorted(batch, np.arange(NGRAPH), side="right")

    src, dst = ei[0], ei[1]
    e_graph = batch[dst]
    e_per_graph = np.bincount(e_graph, minlength=NGRAPH)

    # contiguous graph partition balanced by edge count
    cum = np.cumsum(e_per_graph)
    total = cum[-1]
    cuts = [0]
    for k in range(1, NCORES):
        cuts.append(int(np.searchsorted(cum, total * k / NCORES)))
    cuts.append(NGRAPH)
    g_lo = np.array(cuts[:-1])
    g_hi = np.array(cuts[1:])

    n_lo = np.array([g_start[g_lo[k]] if g_lo[k] < NGRAPH else N for k in range(NCORES)])
    n_hi = np.array([g_end[g_hi[k] - 1] if g_hi[k] > g_lo[k] else n_lo[k] for k in range(NCORES)])
    npc = n_hi - n_lo
    NB = int(np.ceil(npc.max() / 128))
    # AllGather chunk sizes (in 128-node blocks), in block-processing order.
    # Uniform 3-block chunks measured best: bigger head chunks (5) raised
    # mid-layer collective latency and made boundaries worse; the last
    # collective's ~30us is fixed-cost dominated so a smaller tail chunk
    # doesn't help.
    CHUNKS = []
    head = NB
    while head > 0:
        s = min(3, head)
        CHUNKS.append(s)
        head -= s
    NCHUNK = len(CHUNKS)
    chunk_first = np.cumsum([0] + CHUNKS[:-1])          # first block of chunk
    chunk_of_block = np.repeat(np.arange(NCHUNK), CHUNKS)
    table_base = np.cumsum([0] + [NCORES * s * 128 for s in CHUNKS])  # row base
    NPC_PAD = NB * 128
    NPAD_G = int(table_base[-1])
    assert NPAD_G < 32768

    # Renumber nodes within each core so edge counts per 128-node block are
    # balanced (LPT binning by in-degree) — minimizes tile padding (T).
    perms = []
    core_of_node = np.zeros(N, np.int64)
    local_of_node = np.zeros(N, np.int64)
    for k in range(NCORES):
        sl = slice(n_lo[k], n_hi[k])
        core_of_node[sl] = k
        nk = int(npc[k])
        mask = (dst >= n_lo[k]) & (dst < n_hi[k])
        dl0 = dst[np.nonzero(mask)[0]] - n_lo[k]
        deg = np.bincount(dl0, minlength=max(nk, 1))
        order = np.argsort(-deg[:nk], kind="stable")
        perm = np.zeros(max(nk, 1), np.int64)
        bin_sum = np.zeros(NB, np.float64)
        bin_cnt = np.zeros(NB, np.int64)
        for nloc in order:
            cand = np.nonzero(bin_cnt < 128)[0]
            b = cand[np.argmin(bin_sum[cand])]
            perm[nloc] = b * 128 + bin_cnt[b]
            bin_cnt[b] += 1
            bin_sum[b] += deg[nloc]
        perms.append(perm)
        local_of_node[sl] = perm[:nk]
    # Global table row id under the chunked-AllGather layout:
    # U_tabs rows = [chunk][core][node-within-chunk]
    _blk = local_of_node // 128
    _chunk = chunk_of_block[_blk]
    _chunk_nodes = np.array(CHUNKS)[_chunk] * 128
    _within = local_of_node - chunk_first[_chunk] * 128
    gid_of_node = table_base[_chunk] + core_of_node * _chunk_nodes + _within

    # per-core edge lists grouped by dst block
    per_core_edges = []
    blk_counts = np.zeros((NCORES, NB), np.int64)
    for k in range(NCORES):
        mask = (dst >= n_lo[k]) & (dst < n_hi[k])
        eidx = np.nonzero(mask)[0]
        dl = perms[k][dst[eidx] - n_lo[k]]
        order = np.argsort(dl, kind="stable")
        eidx = eidx[order]
        dl = dl[order]
        blocks = dl // 128
        per_blk = [eidx[blocks == b] for b in range(NB)]
        per_core_edges.append(per_blk)
        for b in range(NB):
            blk_counts[k, b] = len(per_blk[b])

    TPB = np.maximum(1, np.ceil(blk_counts.max(axis=0) / 128).astype(np.int64))  # [NB]
    T = int(TPB.sum())
    EPC_PAD = T * 128

    G_MAX = int((g_hi - g_lo).max())
    n_per_graph = g_end - g_start
    assert n_per_graph.max() <= 128, "slot maxpool assumes <=128 nodes/graph"

    cfg = dict(NB=NB, NPC_PAD=NPC_PAD, NPAD_G=NPAD_G, T=T, EPC_PAD=EPC_PAD,
               TPB=tuple(int(t) for t in TPB), G_MAX=G_MAX, n_layers=n_layers,
               CHUNKS=tuple(int(s) for s in CHUNKS))

    def wrap16(idx):
        # [128, len/16] int16, replicated-wrap layout
        n = len(idx)
        assert n % 16 == 0
        w = np.zeros((16, n // 16), np.int16)
        w[np.arange(n) % 16, np.arange(n) // 16] = idx.astype(np.int16)
        return np.tile(w, (8, 1))

    # ---- shared (replicated) tensors ----
    wnode = np.zeros((11, H), np.float32)
    wnode[:10] = np.asarray(inputs["W_node"], np.float32)
    wnode[10] = np.asarray(inputs["b_node"], np.float32)

    wedge = np.zeros((4, H), np.float32)
    wedge[:3] = np.asarray(inputs["W_edge"], np.float32)
    wedge[3] = np.asarray(inputs["b_edge"], np.float32)

    # wfs: [128, nL*768] bf16; per layer i: [dst 256 | src 256 | e 256],
    # each 256 = [Wf part (NEGATED) | Ws part]. bfs: [1, nL*256] (f NEGATED).
    Wf = np.asarray(inputs["Wf"], np.float32)   # [10, 384, 128]
    Ws = np.asarray(inputs["Ws"], np.float32)
    bf = np.asarray(inputs["bf"], np.float32)   # [10, 128]
    bs = np.asarray(inputs["bs"], np.float32)
    wfs = np.zeros((128, n_layers * 768), np.float32)
    bfs = np.zeros((1, n_layers * 256), np.float32)
    for i in range(n_layers):
        for c in range(3):  # 0=dst(x_i) 1=src(x_j) 2=e
            col = i * 768 + c * 256
            wfs[:, col:col + 128] = -Wf[i, c * 128:(c + 1) * 128, :]
            wfs[:, col + 128:col + 256] = Ws[i, c * 128:(c + 1) * 128, :]
        bfs[0, i * 256:i * 256 + 128] = -bf[i]
        bfs[0, i * 256 + 128:(i + 1) * 256] = bs[i]

    bn_g = np.asarray(inputs["bn_g"], np.float64)
    bn_b = np.asarray(inputs["bn_b"], np.float64)
    bn_m = np.asarray(inputs["bn_m"], np.float64)
    bn_v = np.asarray(inputs["bn_v"], np.float64)
    scale = (bn_g / np.sqrt(bn_v + EPS)).astype(np.float32)   # [10, 128]
    shift = (bn_b - bn_m * (bn_g / np.sqrt(bn_v + EPS))).astype(np.float32)
    # feature-major: one column per layer, feature on the partition axis
    # (consumed as per-partition scale/bias APs by the Scalar engine)
    bns = scale[:n_layers].T.copy().astype(np.float32)   # [128, n_layers]
    bnb = shift[:n_layers].T.copy().astype(np.float32)

    iota = np.tile(np.arange(128, dtype=np.float32)[None, :], (128, 1))

    # heads
    W1 = np.asarray(inputs["W1"], np.float64)
    sc1 = (np.asarray(inputs["bn1_g"], np.float64) / np.sqrt(np.asarray(inputs["bn1_v"], np.float64) + EPS))
    sh1 = (np.asarray(inputs["b1"], np.float64) - np.asarray(inputs["bn1_m"], np.float64)) * sc1 + np.asarray(inputs["bn1_b"], np.float64)
    W2 = np.asarray(inputs["W2"], np.float64)
    sc2 = (np.asarray(inputs["bn2_g"], np.float64) / np.sqrt(np.asarray(inputs["bn2_v"], np.float64) + EPS))
    sh2 = (np.asarray(inputs["b2"], np.float64) - np.asarray(inputs["bn2_m"], np.float64)) * sc2 + np.asarray(inputs["bn2_b"], np.float64)
    W3 = np.asarray(inputs["W3"], np.float32)   # [128, 64]
    b3 = np.asarray(inputs["b3"], np.float32)   # [64]
    W4 = np.concatenate([np.asarray(inputs[n], np.float32) for n in ("Wv", "W_en", "Wd", "Wh")], axis=1)  # [64, 4]
    b4 = np.concatenate([np.asarray(inputs[n], np.float32) for n in ("bv", "b_en", "bd", "bh")])  # [4]

    w1p = np.zeros((128, 3 * 256), np.float32)
    for c in range(3):
        w1p[:, c * 256:(c + 1) * 256] = W1[c * 128:(c + 1) * 128, :]
    w2p = np.zeros((128, 2 * 128), np.float32)
    for c in range(2):
        w2p[:, c * 128:(c + 1) * 128] = W2[c * 128:(c + 1) * 128, :]
    w3p = W3.astype(np.float32)
    w4p = np.zeros((64, 4), np.float32)
    w4p[:, :] = W4

    hcol = np.zeros((128, 8), np.float32)
    hcol[:, 0] = sc1[:128]
    hcol[:, 1] = sc1[128:]
    hcol[:, 2] = sh1[:128]
    hcol[:, 3] = sh1[128:]
    hcol[:, 4] = sc2
    hcol[:, 5] = sh2
    hcol[:64, 6] = b3
    hcol[:4, 7] = b4

    shared = dict(wnode=wnode, wedge=wedge, wfs=wfs, bfs=bfs,
                  bns=bns, bnb=bnb, iota=iota, w1p=w1p, w2p=w2p, w3p=w3p,
                  w4p=w4p, hcol=hcol)

    # ---- per-core tensors ----
    in_maps = []
    meta = []
    for k in range(NCORES):
        xT_own = np.zeros((11, NPC_PAD), np.float32)
        xT_own[:10, perms[k][:npc[k]]] = x[n_lo[k]:n_hi[k]].T
        xT_own[10] = 1.0

        eaT = np.zeros((4, EPC_PAD), np.float32)
        eaT[3] = 1.0
        src_ids = np.zeros(EPC_PAD, np.int64)
        dst_rel = np.full(EPC_PAD, -1, np.int64)
        pos = 0
        for b in range(NB):
            eidx = per_core_edges[k][b]
            ne = len(eidx)
            cap = int(TPB[b]) * 128
            assert ne <= cap
            eaT[:3, pos:pos + ne] = ea[eidx].T
            src_ids[pos:pos + ne] = gid_of_node[src[eidx]]
            dst_rel[pos:pos + ne] = perms[k][dst[eidx] - n_lo[k]] - b * 128
            pos += cap
        assert pos == EPC_PAD

        srcg = wrap16(src_ids)

        # onehots: oh1[p=edge-in-tile, t*128 + node] for scatter lhsT;
        #          oh2[p=node, t*128 + edge-in-tile] for dst-part lhsT
        oh1 = np.zeros((128, T * 128), np.float32)
        oh2 = np.zeros((128, T * 128), np.float32)
        tt = np.arange(EPC_PAD) // 128
        pp = np.arange(EPC_PAD) % 128
        valid = dst_rel >= 0
        oh1[pp[valid], tt[valid] * 128 + dst_rel[valid]] = 1.0
        oh2[dst_rel[valid], tt[valid] * 128 + pp[valid]] = 1.0

        invp = np.full(NPC_PAD, -1, np.int64)
        invp[perms[k][:npc[k]]] = np.arange(npc[k])
        grel = np.full((128, NB), -1.0, np.float32)
        for b in range(NB):
            for p in range(128):
                orig = invp[b * 128 + p]
                if orig >= 0:
                    grel[p, b] = float(batch[n_lo[k] + orig] - g_lo[k])

        Gk = int(g_hi[k] - g_lo[k])
        invcnt = np.ones((128, 1), np.float32)
        slot_ids = np.zeros(G_MAX * 128, np.int64)
        for gl in range(G_MAX):
            g = g_lo[k] + gl
            if gl < Gk:
                nodes = np.arange(g_start[g], g_end[g])
                cnt = len(nodes)
                invcnt[gl, 0] = 1.0 / max(cnt, 1)
                sl = perms[k][nodes - n_lo[k]]
                slots = np.resize(sl, 128) if cnt > 0 else np.zeros(128, np.int64)
            else:
                slots = np.zeros(128, np.int64)
            slot_ids[gl * 128:(gl + 1) * 128] = slots
        slotg = wrap16(slot_ids)

        m = dict(shared)
        m.update(xT_own=xT_own, eaT=eaT, srcg=srcg, oh1=oh1, oh2=oh2,
                 grel=grel, invcnt=invcnt, slotg=slotg)
        in_maps.append(m)
        meta.append(dict(g_lo=int(g_lo[k]), g_hi=int(g_hi[k])))

    return in_maps, cfg, meta


# ----------------------------------------------------------------------------
# Bass program
# ----------------------------------------------------------------------------

def _build(cfg, debug_dump=False):
    NB = cfg["NB"]
    NPC_PAD = cfg["NPC_PAD"]
    NPAD_G = cfg["NPAD_G"]
    T = cfg["T"]
    EPC_PAD = cfg["EPC_PAD"]
    TPB = cfg["TPB"]
    G_MAX = cfg["G_MAX"]
    n_layers = cfg["n_layers"]
    CHUNKS = cfg["CHUNKS"]
    NCHUNK = len(CHUNKS)
    chunk_first = [0]
    for s in CHUNKS[:-1]:
        chunk_first.append(chunk_first[-1] + s)
    chunk_of_block = []
    for c, s in enumerate(CHUNKS):
        chunk_of_block += [c] * s
    table_base = [0]
    for s in CHUNKS:
        table_base.append(table_base[-1] + NCORES * s * 128)

    nc = bacc.Bacc("TRN2", debug=False, num_devices=NCORES)

    d_xT_own = nc.dram_tensor("xT_own", [11, NPC_PAD], F32, kind="ExternalInput")
    d_eaT = nc.dram_tensor("eaT", [4, EPC_PAD], F32, kind="ExternalInput")
    d_srcg = nc.dram_tensor("srcg", [128, EPC_PAD // 16], I16, kind="ExternalInput")
    d_oh1 = nc.dram_tensor("oh1", [128, T * 128], F32, kind="ExternalInput")
    d_oh2 = nc.dram_tensor("oh2", [128, T * 128], F32, kind="ExternalInput")
    d_grel = nc.dram_tensor("grel", [128, NB], F32, kind="ExternalInput")
    d_invcnt = nc.dram_tensor("invcnt", [128, 1], F32, kind="ExternalInput")
    d_slotg = nc.dram_tensor("slotg", [128, G_MAX * 128 // 16], I16, kind="ExternalInput")
    d_wnode = nc.dram_tensor("wnode", [11, H], F32, kind="ExternalInput")
    d_wedge = nc.dram_tensor("wedge", [4, H], F32, kind="ExternalInput")
    d_wfs = nc.dram_tensor("wfs", [128, n_layers * 768], F32, kind="ExternalInput")
    d_bfs = nc.dram_tensor("bfs", [1, n_layers * 256], F32, kind="ExternalInput")
    d_bns = nc.dram_tensor("bns", [128, n_layers], F32, kind="ExternalInput")
    d_bnb = nc.dram_tensor("bnb", [128, n_layers], F32, kind="ExternalInput")
    d_iota = nc.dram_tensor("iota", [128, 128], F32, kind="ExternalInput")
    d_w1p = nc.dram_tensor("w1p", [128, 3 * 256], F32, kind="ExternalInput")
    d_w2p = nc.dram_tensor("w2p", [128, 2 * 128], F32, kind="ExternalInput")
    d_w3p = nc.dram_tensor("w3p", [128, 64], F32, kind="ExternalInput")
    d_w4p = nc.dram_tensor("w4p", [64, 4], F32, kind="ExternalInput")
    d_hcol = nc.dram_tensor("hcol", [128, 8], F32, kind="ExternalInput")

    d_out4 = nc.dram_tensor("out4", [4, G_MAX], F32, kind="ExternalOutput")
    if debug_dump:
        d_hdump = nc.dram_tensor("hdump", [NPC_PAD, H], F32, kind="ExternalOutput")

    AF = mybir.ActivationFunctionType
    ALU = mybir.AluOpType

    with tile.TileContext(nc) as tc, nc.allow_low_precision(reason="bf16 conv path; PSUM accumulation stays f32"):
        import contextlib
        ctx = contextlib.ExitStack()
        with ctx:
            cpool = ctx.enter_context(tc.tile_pool(name="const", bufs=1))
            dram = ctx.enter_context(tc.tile_pool(name="dram", bufs=1, space="DRAM"))
            work = ctx.enter_context(tc.tile_pool(name="work", bufs=2))
            gbuf = ctx.enter_context(tc.tile_pool(name="gbuf", bufs=2))
            psum_p = ctx.enter_context(tc.tile_pool(name="psum_p", bufs=2, space="PSUM"))
            psum_a = ctx.enter_context(tc.tile_pool(name="psum_a", bufs=2, space="PSUM"))
            psum_t = ctx.enter_context(tc.tile_pool(name="psum_t", bufs=1, space="PSUM"))

            # DRAM state
            eT_d = dram.tile([128, EPC_PAD], F32, name="eT_d")
            # Per-chunk U staging. Separate tensors so each chunk's AllGather
            # depends only on its own blocks' U writes (collective overlaps
            # remaining compute).
            U_own_c = [dram.tile([CHUNKS[c] * 128, 256], F32, name=f"U_own{c}")
                       for c in range(NCHUNK)]
            # NOTE: plain (Local) DRAM, not addr_space="Shared" — the CoreSim
            # race detector enforces one writer instruction per Shared
            # tensor, which forbids the per-chunk AllGathers. Local-output
            # collectives bounce through per-collective internal buffers.
            U_tabs = []
            for i in range(n_layers):
                U_tabs.append(dram.tile([NPAD_G, 256], F32, name=f"U_tab{i}"))
            hin_slice = dram.tile([NPC_PAD, H], F32, name="hin_slice")

            # constants in SBUF
            ident = cpool.tile([128, 128], F32)
            make_identity(nc, ident[:])
            ones16 = cpool.tile([1, 128], F32)
            nc.vector.memset(ones16[:], 1.0)
            c_wnode = cpool.tile([11, H], F32)
            nc.sync.dma_start(out=c_wnode[:], in_=d_wnode[:])
            c_wedge = cpool.tile([4, H], F32)
            nc.sync.dma_start(out=c_wedge[:], in_=d_wedge[:])
            c_wfs = cpool.tile([128, n_layers * 768], F32)
            nc.sync.dma_start(out=c_wfs[:], in_=d_wfs[:])
            c_bfs = cpool.tile([1, n_layers * 256], F32)
            nc.sync.dma_start(out=c_bfs[:], in_=d_bfs[:])
            c_bns = cpool.tile([128, n_layers], F32)
            nc.sync.dma_start(out=c_bns[:], in_=d_bns[:])
            c_bnb = cpool.tile([128, n_layers], F32)
            nc.sync.dma_start(out=c_bnb[:], in_=d_bnb[:])
            c_iota = cpool.tile([128, 128], F32)
            nc.sync.dma_start(out=c_iota[:], in_=d_iota[:])
            c_srcg = cpool.tile([128, EPC_PAD // 16], I16)
            nc.sync.dma_start(out=c_srcg[:], in_=d_srcg[:])
            c_grel = cpool.tile([128, NB], F32)
            nc.sync.dma_start(out=c_grel[:], in_=d_grel[:])
            c_invcnt = cpool.tile([128, 1], F32)
            nc.sync.dma_start(out=c_invcnt[:], in_=d_invcnt[:])
            c_slotg = cpool.tile([128, G_MAX * 128 // 16], I16)
            nc.sync.dma_start(out=c_slotg[:], in_=d_slotg[:])
            c_w1p = cpool.tile([128, 3 * 256], F32)
            nc.sync.dma_start(out=c_w1p[:], in_=d_w1p[:])
            c_w2p = cpool.tile([128, 2 * 128], F32)
            nc.sync.dma_start(out=c_w2p[:], in_=d_w2p[:])
            c_w3p = cpool.tile([128, 64], F32)
            nc.sync.dma_start(out=c_w3p[:], in_=d_w3p[:])
            c_w4p = cpool.tile([64, 4], F32)
            nc.sync.dma_start(out=c_w4p[:], in_=d_w4p[:])
            c_hcol = cpool.tile([128, 8], F32)
            nc.sync.dma_start(out=c_hcol[:], in_=d_hcol[:])

            # persistent SBUF state.
            # h_own is FEATURE-major: [feat(p), block*128 + node]. This makes
            # h blocks directly usable as matmul lhsT for the U/V tables (no
            # per-block PE transpose + PSUM copy), and turns the BN
            # scale/shift into per-partition ACT scale/bias.
            h_own = cpool.tile([128, NPC_PAD], F32, name="h_own")
            # node-major copy of the final h (for pooling), written once.
            h_nm_all = cpool.tile([128, NPC_PAD], F32, name="h_nm_all")
            V_all = cpool.tile([128, NB * 256], F32, name="V_all")

            def emit_block_post(i_next, b):
                """After h_own block b is final: compute V (dst) and U (src)
                tables for layer i_next; DMA U slice to its chunk's staging
                buffer; when the chunk is complete, AllGather it into
                U_tabs[i_next] (overlapping remaining blocks' compute)."""
                bs_ = slice(b * 128, (b + 1) * 128)
                # V (dst part) | U (src part) in one N=512 matmul; bias rides
                # a K=1 accumulate into the U half. h_own is feature-major so
                # the h block IS the lhsT — no transpose needed.
                pall = psum_t.tile([128, 512], F32, tag="uv", bufs=2)
                nc.tensor.matmul(out=pall[:], lhsT=h_own[:, bs_],
                                 rhs=c_wfs[:, i_next * 768:i_next * 768 + 512],
                                 start=True, stop=False)
                nc.tensor.matmul(out=pall[:, 256:512], lhsT=ones16[:],
                                 rhs=c_bfs[:, i_next * 256:(i_next + 1) * 256],
                                 start=False, stop=True)
                nc.vector.tensor_copy(out=V_all[:, b * 256:(b + 1) * 256],
                                      in_=pall[:, 0:256])
                u32 = work.tile([128, 256], F32, tag="u32")
                nc.scalar.copy(out=u32[:], in_=pall[:, 256:512])
                c = chunk_of_block[b]
                boff = b - chunk_first[c]
                nc.sync.dma_start(
                    out=U_own_c[c][boff * 128:(boff + 1) * 128, :],
                    in_=u32[:])
                if b - chunk_first[c] == CHUNKS[c] - 1:
                    nc.gpsimd.collective_compute(
                        "AllGather", ALU.bypass,
                        replica_groups=[list(range(NCORES))],
                        ins=[U_own_c[c].opt()],
                        outs=[U_tabs[i_next][table_base[c]:table_base[c + 1], :].opt()])

            with tc.tile_pool(name="enc", bufs=1) as enc:
                # ---------------- encoder: own nodes (first, so the layer-0
                # U AllGather chunks launch ASAP and overlap the edge
                # encoder) ----------------
                for b in range(NB):
                    xo_sb = enc.tile([11, 128], F32, tag="xo_sb", bufs=2)
                    nc.sync.dma_start(out=xo_sb[:], in_=d_xT_own[:, b * 128:(b + 1) * 128])
                    ph = psum_a.tile([128, 128], F32, tag="agg")
                    # lhsT=W, rhs=x -> out [feat, node] (feature-major h)
                    nc.tensor.matmul(out=ph[:], lhsT=c_wnode[:],
                                     rhs=xo_sb[:], start=True, stop=True)
                    nc.scalar.activation(h_own[:, b * 128:(b + 1) * 128], ph[:], AF.Relu)
                    emit_block_post(0, b)

                # ---------------- encoder: edges ----------------
                # Triple-buffered: with bufs=1 the DMA->MM->relu->DMA chain
                # fully serialized (~3.5us/chunk, ~300us startup stall before
                # layer 0's first slab).
                for c in range(0, EPC_PAD, 256):
                    w = min(256, EPC_PAD - c)
                    ea_sb = enc.tile([4, 256], F32, tag="ea_sb", bufs=3)
                    nc.sync.dma_start(out=ea_sb[:, :w], in_=d_eaT[:, c:c + w])
                    pe = psum_p.tile([128, SLAB, 256], F32, tag="P")
                    nc.tensor.matmul(out=pe[:, 0, :w],
                                     lhsT=c_wedge[:], rhs=ea_sb[:, :w],
                                     start=True, stop=True)
                    et_sb = enc.tile([128, 256], F32, tag="et_sb", bufs=2)
                    nc.scalar.activation(et_sb[:, :w], pe[:, 0, :w], AF.Relu)
                    nc.sync.dma_start(out=eT_d[:, c:c + w], in_=et_sb[:, :w])

            # tile index -> block
            tile_block = []
            for b in range(NB):
                tile_block += [b] * TPB[b]
            first_tile_of_block = {}
            last_tile_of_block = {}
            for t, b in enumerate(tile_block):
                if b not in first_tile_of_block:
                    first_tile_of_block[b] = t
                last_tile_of_block[b] = t

            n_gch = (EPC_PAD + GCH - 1) // GCH
            n_slab = (T + SLAB - 1) // SLAB

            def block_finish(i, b, agg):
                # agg is [feat, node] (scatter matmul emits feature-major);
                # BN scale/shift are per-feature = per-partition, fused into
                # the ACT Relu as scale/bias.
                bs_ = slice(b * 128, (b + 1) * 128)
                t0 = work.tile([128, 128], F32, tag="t0")
                nc.vector.tensor_tensor(out=t0[:], in0=agg[:], in1=h_own[:, bs_],
                                        op=ALU.add)
                if i % 2 == 1:
                    t3 = work.tile([128, 128], F32, tag="t3")
                    nc.scalar.activation(t3[:], t0[:], AF.Relu,
                                         bias=c_bnb[:, i:i + 1],
                                         scale=c_bns[:, i:i + 1])
                    nc.vector.tensor_tensor(out=h_own[:, bs_], in0=t3[:],
                                            in1=h_own[:, bs_], op=ALU.add)
                else:
                    nc.scalar.activation(h_own[:, bs_], t0[:], AF.Relu,
                                         bias=c_bnb[:, i:i + 1],
                                         scale=c_bns[:, i:i + 1])
                if i < n_layers - 1:
                    emit_block_post(i + 1, b)
                else:
                    # node-major copy for pooling + the maxpool slot gather.
                    # (psum_a "agg" tag: only live at the last layer, so it
                    # shares banks with the scatter aggregators instead of
                    # costing psum_t a dedicated bank.)
                    tp = psum_a.tile([128, 128], F32, tag="agg")
                    nc.tensor.transpose(out=tp[:], in_=h_own[:, bs_],
                                        identity=ident[:])
                    nc.scalar.copy(out=h_nm_all[:, bs_], in_=tp[:])
                    nc.sync.dma_start(out=hin_slice[b * 128:(b + 1) * 128, :],
                                      in_=h_nm_all[:, bs_])

            # ---------------- conv layers ----------------
            for i in range(n_layers):
                u_t = []
                eTb_t = []
                oh1_t = []
                oh2_t = []
                for c in range(n_gch):
                    lo = c * GCH
                    hi = min(EPC_PAD, lo + GCH)
                    w = hi - lo
                    ug = gbuf.tile([128, GCH // 128, 256], F32, tag="ug", bufs=3)
                    nc.gpsimd.dma_gather(
                        out_ap=ug[:, :w // 128, :], in_ap=U_tabs[i][:],
                        idxs_ap=c_srcg[:, lo // 16:hi // 16],
                        num_idxs=w, num_idxs_reg=w, elem_size=256)
                    eTb = gbuf.tile([128, GCH], F32, tag="eTb", bufs=2)
                    nc.sync.dma_start(out=eTb[:, :w], in_=eT_d[:, lo:hi])
                    oh1b = gbuf.tile([128, GCH], F32, tag="oh1b", bufs=2)
                    nc.sync.dma_start(out=oh1b[:, :w], in_=d_oh1[:, lo:hi])
                    oh2b = gbuf.tile([128, GCH], F32, tag="oh2b", bufs=2)
                    nc.sync.dma_start(out=oh2b[:, :w], in_=d_oh2[:, lo:hi])
                    u_t.append(ug)
                    eTb_t.append(eTb)
                    oh1_t.append(oh1b)
                    oh2_t.append(oh2b)

                wcol = i * 768
                pend = []  # (tiles, MSG tile) awaiting scatter

                def emit_scatter(tiles, MSG, aggs, i=i):
                    for j, t in enumerate(tiles):
                        b = tile_block[t]
                        ch, off = t * 128 // GCH, (t * 128 % GCH) // 128
                        if t == first_tile_of_block[b]:
                            aggs[b] = psum_a.tile([128, 128], F32, tag="agg",
                                                  name="agg")
                        # lhsT=MSG, rhs=onehot -> agg [feat, node]: same
                        # products/accumulation order as the node-major form
                        # (contraction over the same 128 edges), but the
                        # output lands feature-major for free.
                        nc.tensor.matmul(
                            out=aggs[b][:],
                            lhsT=MSG[:, j * 128:(j + 1) * 128],
                            rhs=oh1_t[ch][:, off * 128:(off + 1) * 128],
                            start=(t == first_tile_of_block[b]),
                            stop=(t == last_tile_of_block[b]))
                        if t == last_tile_of_block[b]:
                            block_finish(i, b, aggs.pop(b))

                aggs = {}
                for s in range(n_slab):
                    t0_ = s * SLAB
                    tiles = list(range(t0_, min(T, t0_ + SLAB)))
                    nj = len(tiles)
                    P = psum_p.tile([128, SLAB, 256], F32, tag="P")
                    for j, t in enumerate(tiles):
                        b = tile_block[t]
                        ch, off = t * 128 // GCH, (t * 128 % GCH) // 128
                        nc.tensor.matmul(out=P[:, j, :],
                                         lhsT=oh2_t[ch][:, off * 128:(off + 1) * 128],
                                         rhs=V_all[:, b * 256:(b + 1) * 256],
                                         start=True, stop=False)
                        nc.tensor.matmul(out=P[:, j, :],
                                         lhsT=eTb_t[ch][:, off * 128:(off + 1) * 128],
                                         rhs=c_wfs[:, wcol + 512:wcol + 768],
                                         start=False, stop=True)
                    ch0 = t0_ * 128 // GCH
                    off0 = (t0_ * 128 % GCH) // 128
                    w1 = nj * 128

                    # FS de-interleaved: FS[:,0,:]=-f, FS[:,1,:]=s — two half
                    # adds pay the strided PSUM/u read once so every later
                    # elementwise op runs on contiguous SBUF. f32: exp args
                    # must not be bf16-rounded — values reach +-40k.
                    # Fresh-output tiles for the tensor_scalar ops; fused
                    # 2-wide Exp (instruction count beats per-op density).
                    FS = work.tile([128, 2, SLAB * 128], F32, tag="FS")
                    nc.vector.tensor_tensor(
                        out=FS[:, 0, :w1], in0=P[:, :nj, 0:128],
                        in1=u_t[ch0][:, off0:off0 + nj, 0:128], op=ALU.add)
                    nc.vector.tensor_tensor(
                        out=FS[:, 1, :w1], in0=P[:, :nj, 128:256],
                        in1=u_t[ch0][:, off0:off0 + nj, 128:256], op=ALU.add)
                    # RS = max(s, 0) on the (less busy) Scalar engine
                    RS = work.tile([128, SLAB * 128], F32, tag="RS")
                    nc.scalar.activation(RS[:, :w1], FS[:, 1, :w1], AF.Relu)
                    # G[:,0] = clamp(-f, +-30); G[:,1] = max(-|s|, -30)
                    # (Exp LUT yields NaN/garbage for out-of-range args)
                    G = work.tile([128, 2, SLAB * 128], F32, tag="G")
                    nc.vector.tensor_scalar(
                        out=G[:, 0, :w1], in0=FS[:, 0, :w1],
                        scalar1=-CLAMP, scalar2=CLAMP, op0=ALU.max, op1=ALU.min)
                    T1 = work.tile([128, SLAB * 128], F32, tag="T1", bufs=1)
                    nc.vector.scalar_tensor_tensor(
                        out=T1[:, :w1], in0=FS[:, 1, :w1], scalar=0.0,
                        in1=RS[:, :w1], op0=ALU.min, op1=ALU.subtract)
                    nc.vector.tensor_scalar_max(out=G[:, 1, :w1],
                                                in0=T1[:, :w1], scalar1=-CLAMP)
                    # E = exp(G) (one fused 2-region pass); L = ln(1+E) split
                    # so the sigmoid path stays f32: L_f = ln(1+e^-f) = -ln(sig)
                    E = work.tile([128, 2, SLAB * 128], F32, tag="E", bufs=1)
                    nc.scalar.activation(E[:, :, :w1], G[:, :, :w1], AF.Exp)
                    LF = work.tile([128, SLAB * 128], F32, tag="LF", bufs=1)
                    nc.scalar.activation(LF[:, :w1], E[:, 0, :w1], AF.Ln, bias=1.0)
                    LS = work.tile([128, SLAB * 128], F32, tag="LS")
                    nc.scalar.activation(LS[:, :w1], E[:, 1, :w1], AF.Ln, bias=1.0)
                    SG = work.tile([128, SLAB * 128], F32, tag="SG")
                    nc.scalar.activation(SG[:, :w1], LF[:, :w1], AF.Exp, scale=-1.0)
                    SP = work.tile([128, SLAB * 128], F32, tag="SP")
                    nc.vector.tensor_tensor(out=SP[:, :w1], in0=RS[:, :w1],
                                            in1=LS[:, :w1], op=ALU.add)
                    MSG = work.tile([128, SLAB * 128], F32, tag="MSG")
                    nc.vector.tensor_tensor(out=MSG[:, :w1], in0=SP[:, :w1],
                                            in1=SG[:, :w1], op=ALU.mult)
                    pend.append((tiles, MSG))
                    if len(pend) > 1:
                        emit_scatter(*pend.pop(0), aggs)
                while pend:
                    emit_scatter(*pend.pop(0), aggs)
                # (U exchange for the next layer is issued per-chunk inside
                # emit_block_post, overlapping the rest of this layer.)

            if debug_dump:
                for b in range(NB):
                    nc.sync.dma_start(out=d_hdump[b * 128:(b + 1) * 128, :],
                                      in_=h_nm_all[:, b * 128:(b + 1) * 128])

            # ---------------- pooling ----------------
            ppool = psum_a.tile([128, 128], F32, tag="agg")
            for b in range(NB):
                ohg = work.tile([128, 128], F32, tag="ohg")
                nc.vector.tensor_tensor(
                    out=ohg[:], in0=c_grel[:, b:b + 1].to_broadcast([128, 128]),
                    in1=c_iota[:], op=ALU.is_equal)
                nc.tensor.matmul(out=ppool[:], lhsT=ohg[:],
                                 rhs=h_nm_all[:, b * 128:(b + 1) * 128],
                                 start=(b == 0), stop=(b == NB - 1))
            sum_nm = work.tile([128, 128], F32, tag="sum_nm")
            nc.vector.tensor_copy(out=sum_nm[:], in_=ppool[:])
            mean_nm = work.tile([128, 128], F32, tag="mean_nm")
            nc.scalar.activation(mean_nm[:], ppool[:], AF.Identity, scale=c_invcnt[:])

            gT = cpool.tile([128, 3 * G_MAX], F32, name="gT")
            pt = psum_a.tile([128, 128], F32, tag="agg")
            nc.tensor.transpose(out=pt[:], in_=mean_nm[:], identity=ident[:])
            nc.scalar.copy(out=gT[:, 0:G_MAX], in_=pt[:, 0:G_MAX])
            pt2 = psum_a.tile([128, 128], F32, tag="agg")
            nc.tensor.transpose(out=pt2[:], in_=sum_nm[:], identity=ident[:])
            nc.scalar.copy(out=gT[:, 2 * G_MAX:3 * G_MAX], in_=pt2[:, 0:G_MAX])

            # max pool via slot gather
            n_sch = (G_MAX * 128 + GCH - 1) // GCH
            gslot_t = []
            for c in range(n_sch):
                lo = c * GCH
                hi = min(G_MAX * 128, lo + GCH)
                w = hi - lo
                gslot = gbuf.tile([128, GCH // 128, H], F32, tag="gslot")
                nc.gpsimd.dma_gather(
                    out_ap=gslot[:, :w // 128, :], in_ap=hin_slice[:],
                    idxs_ap=c_slotg[:, lo // 16:hi // 16],
                    num_idxs=w, num_idxs_reg=w, elem_size=H)
                gslot_t.append(gslot)
            for g in range(G_MAX):
                ch, off = g * 128 // GCH, (g * 128 % GCH) // 128
                ptm = psum_a.tile([128, 128], F32, tag="agg")
                nc.tensor.transpose(out=ptm[:], in_=gslot_t[ch][:, off, :], identity=ident[:])
                nc.vector.reduce_max(out=gT[:, G_MAX + g:G_MAX + g + 1], in_=ptm[:],
                                     axis=mybir.AxisListType.X)

            # ---------------- heads ----------------
            p1a = psum_p.tile([128, SLAB, 256], F32, tag="P")
            p1b = psum_p.tile([128, SLAB, 256], F32, tag="P")
            for c in range(3):
                rhs = gT[:, c * G_MAX:(c + 1) * G_MAX]
                nc.tensor.matmul(out=p1a[:, 0, :G_MAX], lhsT=c_w1p[:, c * 256:c * 256 + 128],
                                 rhs=rhs, start=(c == 0), stop=(c == 2))
                nc.tensor.matmul(out=p1b[:, 0, :G_MAX], lhsT=c_w1p[:, c * 256 + 128:(c + 1) * 256],
                                 rhs=rhs, start=(c == 0), stop=(c == 2))
            g1a = work.tile([128, G_MAX], F32, tag="g1a")
            nc.scalar.activation(g1a[:], p1a[:, 0, :G_MAX], AF.Relu, bias=c_hcol[:, 2:3], scale=c_hcol[:, 0:1])
            g1b = work.tile([128, G_MAX], F32, tag="g1b")
            nc.scalar.activation(g1b[:], p1b[:, 0, :G_MAX], AF.Relu, bias=c_hcol[:, 3:4], scale=c_hcol[:, 1:2])

            p2 = psum_p.tile([128, SLAB, 256], F32, tag="P")
            nc.tensor.matmul(out=p2[:, 0, :G_MAX], lhsT=c_w2p[:, 0:128], rhs=g1a[:], start=True, stop=False)
            nc.tensor.matmul(out=p2[:, 0, :G_MAX], lhsT=c_w2p[:, 128:256], rhs=g1b[:], start=False, stop=True)
            g2 = work.tile([128, G_MAX], F32, tag="g2")
            nc.scalar.activation(g2[:], p2[:, 0, :G_MAX], AF.Relu, bias=c_hcol[:, 5:6], scale=c_hcol[:, 4:5])

            p3 = psum_p.tile([128, SLAB, 256], F32, tag="P")
            nc.tensor.matmul(out=p3[:64, 0, :G_MAX], lhsT=c_w3p[:], rhs=g2[:], start=True, stop=True)
            g3 = work.tile([64, G_MAX], F32, tag="g3")
            nc.scalar.activation(g3[:], p3[:64, 0, :G_MAX], AF.Relu, bias=c_hcol[:64, 6:7])

            p4 = psum_p.tile([128, SLAB, 256], F32, tag="P")
            nc.tensor.matmul(out=p4[:4, 0, :G_MAX], lhsT=c_w4p[:], rhs=g3[:], start=True, stop=True)
            o4 = work.tile([4, G_MAX], F32, tag="o4")
            nc.scalar.activation(o4[:], p4[:4, 0, :G_MAX], AF.Identity, bias=c_hcol[:4, 7:8])
            nc.sync.dma_start(out=d_out4[:], in_=o4[:])

    nc.compile()
    return nc


# ----------------------------------------------------------------------------
# Entry point
# ----------------------------------------------------------------------------

_CACHE = {}


def kernel(trace=False, n_layers=NLAYERS, debug_dump=False, **inputs):
    in_maps, cfg, meta = _prepare(inputs, n_layers=n_layers)
    key = (tuple(sorted(cfg.items())), debug_dump)
    if key not in _CACHE:
        _CACHE[key] = _build(cfg, debug_dump=debug_dump)
    nc = _CACHE[key]

    res = run_bass_kernel_spmd(nc, in_maps, core_ids=list(range(NCORES)), trace=trace)

    outs = [np.zeros((NGRAPH, 1), np.float32) for _ in range(4)]
    for k in range(NCORES):
        g_lo, g_hi = meta[k]["g_lo"], meta[k]["g_hi"]
        o4 = res.results[k]["out4"]   # [4, G_MAX]
        for j in range(4):
            outs[j][g_lo:g_hi, 0] = o4[j, :g_hi - g_lo]
    kernel._last_res = res
    if debug_dump:
        kernel._last_hdump = [res.results[k]["hdump"] for k in range(NCORES)]
        kernel._last_cfg = cfg
    return tuple(outs)



# revision 67
# speedup vs baseline: 1.0849x; 1.0268x over previous
"""Trainium2 Bass kernel for nn_BatteryGNN (CGConv message-passing GNN).

Self-contained: takes full inputs, shards graph-data-parallel across 8
NeuronCores, runs a single SPMD NEFF (10 CGConv layers + pooling + MLP heads),
gathers per-core head outputs on the host.

Design (vs original baseline):
- Per-edge work restructured around a per-layer "U table":
  U[n] = h[n] @ [Wf_src | Ws_src] + [bf | bs]  (256 wide, f32),
  computed per owned 128-node block, all-gathered, then ONE dma_gather per
  edge chunk fetches U[src] rows directly in [edge, 256] pre-act layout.
  This kills the baseline's dst gather (halves the SWDGE gather count),
  both per-tile PE transposes, their PSUM->SBUF copies, and the per-tile
  bias matmul.
- The U AllGather is CHUNKED (3 node-blocks per collective, issued as each
  chunk's U lands): overlaps the collective with the remaining layer
  compute instead of a ~120us full-pipeline stall per layer boundary.
  U_tabs are plain (Local) DRAM because CoreSim enforces a single writer
  instruction per Shared tensor.
- h is kept FEATURE-major ([feat(p), node]): h blocks are directly the
  lhsT for the U/V matmuls (no per-block PE transpose+copy per layer), the
  BN scale/shift fold into per-partition ACT Relu scale/bias, and the
  scatter matmul (lhsT=MSG, rhs=onehot) emits the aggregate feature-major
  for free. One transpose per block after the LAST layer restores
  node-major h for pooling.
- dst-part contribution via host-precomputed onehot matmuls against a local
  per-block V table (V[n] = h[n] @ Wf_dst, no gather, no collective);
  onehots are streamed from DRAM per layer (too big for SBUF in f32).
- Scatter aggregation via host-precomputed onehots (no per-tile DVE
  is_equal).
- sigmoid*softplus computed slab-wide (4 tiles at a time) with 5 ACT passes
  (Relu, fused 2-wide Exp, Ln, Ln, Exp) + 7 DVE passes incl. fused
  scalar_tensor_tensor, all with fresh-output tiles. f pre-acts are negated
  via negated Wf/bf so the sigmoid needs no reciprocal:
  sig = exp(-ln(1+e^-f)).
- Encoder runs node-encode FIRST (feeds the layer-0 AllGather chunks),
  then the edge encoder (triple-buffered DMA) overlaps those collectives.
- The whole conv value path MUST be f32: the network amplifies per-layer
  relative error by ~1e5 (sigmoid gates flip), so bf16/f32r/fp16 anywhere
  in h/U/V/msg blows past the 2e-2 gate (verified empirically).
- Pre-act clamps to +-30 before Exp are mandatory: the Exp LUT returns
  garbage/NaN for far-out-of-range arguments (verified on HW).
"""
import sys

sys.path.insert(0, "/opt/trn_rl_repo")

import numpy as np
import ml_dtypes

import concourse.bacc as bacc
import concourse.bass as bass
import concourse.mybir as mybir
import concourse.tile as tile
from concourse.bass_utils import run_bass_kernel_spmd
from concourse.masks import make_identity

F32 = mybir.dt.float32
BF16 = mybir.dt.bfloat16
I16 = mybir.dt.int16
I8 = mybir.dt.int8
F32R = mybir.dt.float32r
NPBF = ml_dtypes.bfloat16

# Pin every ACT op to the one LUT set containing all functions we use
# (Exp, Ln, Relu, Copy, Identity). Without this, the table chooser can
# alternate tables, inserting ~1.3us ACT_TABLE_LOADs.
_orig_get_act_tables = bacc.get_activation_tables


def _pinned_act_tables(module_arch):
    tabs = dict(_orig_get_act_tables(module_arch))
    keep = "natural_log_exp_and_others"
    ours = {
        mybir.ActivationFunctionType.Exp,
        mybir.ActivationFunctionType.Ln,
        mybir.ActivationFunctionType.Relu,
        mybir.ActivationFunctionType.Copy,
        mybir.ActivationFunctionType.Identity,
    }
    out = {}
    for name, fns in tabs.items():
        out[name] = set(fns) if name == keep else (set(fns) - ours)
    return out


bacc.get_activation_tables = _pinned_act_tables

NCORES = 8
H = 128
NGRAPH = 256
EPS = 1e-5
NLAYERS = 10
GCH = 1024       # edges per bulk-gather chunk (>1024 hangs the SWDGE gather ucode)
SLAB = 4         # tiles per elementwise slab (SLAB*256 f32 = 2 PSUM banks)
CLAMP = 30.0     # pre-act clamp before Exp


# ----------------------------------------------------------------------------
# Host-side preprocessing
# ----------------------------------------------------------------------------

def _prepare(inputs, n_layers=NLAYERS):
    x = np.asarray(inputs["x"], np.float32)              # [N, 10]
    ea = np.asarray(inputs["edge_attr"], np.float32)     # [E, 3]
    ei = np.asarray(inputs["edge_index"]).astype(np.int64)  # [2, E]
    batch = np.asarray(inputs["batch"]).astype(np.int64)    # [N] sorted
    N, E = x.shape[0], ea.shape[0]

    # graph -> node range (batch sorted)
    g_start = np.searchsorted(batch, np.arange(NGRAPH), side="left")
    g_end = np.searchsorted(batch, np.arange(NGRAPH), side="right")

    src, dst = ei[0], ei[1]
    e_graph = batch[dst]
    e_per_graph = np.bincount(e_graph, minlength=NGRAPH)

    # contiguous graph partition balanced by edge count
    cum = np.cumsum(e_per_graph)
    total = cum[-1]
    cuts = [0]
    for k in range(1, NCORES):
        cuts.append(int(np.searchsorted(cum, total * k / NCORES)))
    cuts.append(NGRAPH)
    g_lo = np.array(cuts[:-1])
    g_hi = np.array(cuts[1:])

    n_lo = np.array([g_start[g_lo[k]] if g_lo[k] < NGRAPH else N for k in range(NCORES)])
    n_hi = np.array([g_end[g_hi[k] - 1] if g_hi[k] > g_lo[k] else n_lo[k] for k in range(NCORES)])
    npc = n_hi - n_lo
    NB = int(np.ceil(npc.max() / 128))
    # AllGather chunk sizes (in 128-node blocks), in block-processing order.
    # Uniform 3-block chunks measured best: bigger head chunks (5) raised
    # mid-layer collective latency, and single-block tail chunks (tried to
    # shrink the exposed last collective) also regressed — per-collective
    # fixed cost and CC queueing dominate small transfers.
    CHUNKS = []
    head = NB
    while head > 0:
        s = min(3, head)
        CHUNKS.append(s)
        head -= s
    NCHUNK = len(CHUNKS)
    chunk_first = np.cumsum([0] + CHUNKS[:-1])          # first block of chunk
    chunk_of_block = np.repeat(np.arange(NCHUNK), CHUNKS)
    table_base = np.cumsum([0] + [NCORES * s * 128 for s in CHUNKS])  # row base
    NPC_PAD = NB * 128
    NPAD_G = int(table_base[-1])
    assert NPAD_G < 32768

    # Renumber nodes within each core so edge counts per 128-node block are
    # balanced (LPT binning by in-degree) — minimizes tile padding (T).
    perms = []
    core_of_node = np.zeros(N, np.int64)
    local_of_node = np.zeros(N, np.int64)
    for k in range(NCORES):
        sl = slice(n_lo[k], n_hi[k])
        core_of_node[sl] = k
        nk = int(npc[k])
        mask = (dst >= n_lo[k]) & (dst < n_hi[k])
        dl0 = dst[np.nonzero(mask)[0]] - n_lo[k]
        deg = np.bincount(dl0, minlength=max(nk, 1))
        order = np.argsort(-deg[:nk], kind="stable")
        perm = np.zeros(max(nk, 1), np.int64)
        bin_sum = np.zeros(NB, np.float64)
        bin_cnt = np.zeros(NB, np.int64)
        for nloc in order:
            cand = np.nonzero(bin_cnt < 128)[0]
            b = cand[np.argmin(bin_sum[cand])]
            perm[nloc] = b * 128 + bin_cnt[b]
            bin_cnt[b] += 1
            bin_sum[b] += deg[nloc]
        perms.append(perm)
        local_of_node[sl] = perm[:nk]
    # Global table row id under the chunked-AllGather layout:
    # U_tabs rows = [chunk][core][node-within-chunk]
    _blk = local_of_node // 128
    _chunk = chunk_of_block[_blk]
    _chunk_nodes = np.array(CHUNKS)[_chunk] * 128
    _within = local_of_node - chunk_first[_chunk] * 128
    gid_of_node = table_base[_chunk] + core_of_node * _chunk_nodes + _within

    # per-core edge lists grouped by dst block
    per_core_edges = []
    blk_counts = np.zeros((NCORES, NB), np.int64)
    for k in range(NCORES):
        mask = (dst >= n_lo[k]) & (dst < n_hi[k])
        eidx = np.nonzero(mask)[0]
        dl = perms[k][dst[eidx] - n_lo[k]]
        order = np.argsort(dl, kind="stable")
        eidx = eidx[order]
        dl = dl[order]
        blocks = dl // 128
        per_blk = [eidx[blocks == b] for b in range(NB)]
        per_core_edges.append(per_blk)
        for b in range(NB):
            blk_counts[k, b] = len(per_blk[b])

    TPB = np.maximum(1, np.ceil(blk_counts.max(axis=0) / 128).astype(np.int64))  # [NB]
    T = int(TPB.sum())
    EPC_PAD = T * 128

    G_MAX = int((g_hi - g_lo).max())
    n_per_graph = g_end - g_start
    assert n_per_graph.max() <= 128, "slot maxpool assumes <=128 nodes/graph"

    cfg = dict(NB=NB, NPC_PAD=NPC_PAD, NPAD_G=NPAD_G, T=T, EPC_PAD=EPC_PAD,
               TPB=tuple(int(t) for t in TPB), G_MAX=G_MAX, n_layers=n_layers,
               CHUNKS=tuple(int(s) for s in CHUNKS))

    def wrap16(idx):
        # [128, len/16] int16, replicated-wrap layout
        n = len(idx)
        assert n % 16 == 0
        w = np.zeros((16, n // 16), np.int16)
        w[np.arange(n) % 16, np.arange(n) // 16] = idx.astype(np.int16)
        return np.tile(w, (8, 1))

    # ---- shared (replicated) tensors ----
    wnode = np.zeros((11, H), np.float32)
    wnode[:10] = np.asarray(inputs["W_node"], np.float32)
    wnode[10] = np.asarray(inputs["b_node"], np.float32)

    wedge = np.zeros((4, H), np.float32)
    wedge[:3] = np.asarray(inputs["W_edge"], np.float32)
    wedge[3] = np.asarray(inputs["b_edge"], np.float32)

    # wfs: [128, nL*768] bf16; per layer i: [dst 256 | src 256 | e 256],
    # each 256 = [Wf part (NEGATED) | Ws part]. bfs: [1, nL*256] (f NEGATED).
    Wf = np.asarray(inputs["Wf"], np.float32)   # [10, 384, 128]
    Ws = np.asarray(inputs["Ws"], np.float32)
    bf = np.asarray(inputs["bf"], np.float32)   # [10, 128]
    bs = np.asarray(inputs["bs"], np.float32)
    wfs = np.zeros((128, n_layers * 768), np.float32)
    bfs = np.zeros((1, n_layers * 256), np.float32)
    for i in range(n_layers):
        for c in range(3):  # 0=dst(x_i) 1=src(x_j) 2=e
            col = i * 768 + c * 256
            wfs[:, col:col + 128] = -Wf[i, c * 128:(c + 1) * 128, :]
            wfs[:, col + 128:col + 256] = Ws[i, c * 128:(c + 1) * 128, :]
        bfs[0, i * 256:i * 256 + 128] = -bf[i]
        bfs[0, i * 256 + 128:(i + 1) * 256] = bs[i]

    bn_g = np.asarray(inputs["bn_g"], np.float64)
    bn_b = np.asarray(inputs["bn_b"], np.float64)
    bn_m = np.asarray(inputs["bn_m"], np.float64)
    bn_v = np.asarray(inputs["bn_v"], np.float64)
    scale = (bn_g / np.sqrt(bn_v + EPS)).astype(np.float32)   # [10, 128]
    shift = (bn_b - bn_m * (bn_g / np.sqrt(bn_v + EPS))).astype(np.float32)
    # feature-major: one column per layer, feature on the partition axis
    # (consumed as per-partition scale/bias APs by the Scalar engine)
    bns = scale[:n_layers].T.copy().astype(np.float32)   # [128, n_layers]
    bnb = shift[:n_layers].T.copy().astype(np.float32)

    iota = np.tile(np.arange(128, dtype=np.float32)[None, :], (128, 1))

    # heads
    W1 = np.asarray(inputs["W1"], np.float64)
    sc1 = (np.asarray(inputs["bn1_g"], np.float64) / np.sqrt(np.asarray(inputs["bn1_v"], np.float64) + EPS))
    sh1 = (np.asarray(inputs["b1"], np.float64) - np.asarray(inputs["bn1_m"], np.float64)) * sc1 + np.asarray(inputs["bn1_b"], np.float64)
    W2 = np.asarray(inputs["W2"], np.float64)
    sc2 = (np.asarray(inputs["bn2_g"], np.float64) / np.sqrt(np.asarray(inputs["bn2_v"], np.float64) + EPS))
    sh2 = (np.asarray(inputs["b2"], np.float64) - np.asarray(inputs["bn2_m"], np.float64)) * sc2 + np.asarray(inputs["bn2_b"], np.float64)
    W3 = np.asarray(inputs["W3"], np.float32)   # [128, 64]
    b3 = np.asarray(inputs["b3"], np.float32)   # [64]
    W4 = np.concatenate([np.asarray(inputs[n], np.float32) for n in ("Wv", "W_en", "Wd", "Wh")], axis=1)  # [64, 4]
    b4 = np.concatenate([np.asarray(inputs[n], np.float32) for n in ("bv", "b_en", "bd", "bh")])  # [4]

    w1p = np.zeros((128, 3 * 256), np.float32)
    for c in range(3):
        w1p[:, c * 256:(c + 1) * 256] = W1[c * 128:(c + 1) * 128, :]
    w2p = np.zeros((128, 2 * 128), np.float32)
    for c in range(2):
        w2p[:, c * 128:(c + 1) * 128] = W2[c * 128:(c + 1) * 128, :]
    w3p = W3.astype(np.float32)
    w4p = np.zeros((64, 4), np.float32)
    w4p[:, :] = W4

    hcol = np.zeros((128, 8), np.float32)
    hcol[:, 0] = sc1[:128]
    hcol[:, 1] = sc1[128:]
    hcol[:, 2] = sh1[:128]
    hcol[:, 3] = sh1[128:]
    hcol[:, 4] = sc2
    hcol[:, 5] = sh2
    hcol[:64, 6] = b3
    hcol[:4, 7] = b4

    shared = dict(wnode=wnode, wedge=wedge, wfs=wfs, bfs=bfs,
                  bns=bns, bnb=bnb, iota=iota, w1p=w1p, w2p=w2p, w3p=w3p,
                  w4p=w4p, hcol=hcol)

    # ---- per-core tensors ----
    in_maps = []
    meta = []
    for k in range(NCORES):
        xT_own = np.zeros((11, NPC_PAD), np.float32)
        xT_own[:10, perms[k][:npc[k]]] = x[n_lo[k]:n_hi[k]].T
        xT_own[10] = 1.0

        eaT = np.zeros((4, EPC_PAD), np.float32)
        eaT[3] = 1.0
        src_ids = np.zeros(EPC_PAD, np.int64)
        dst_rel = np.full(EPC_PAD, -1, np.int64)
        pos = 0
        for b in range(NB):
            eidx = per_core_edges[k][b]
            ne = len(eidx)
            cap = int(TPB[b]) * 128
            assert ne <= cap
            eaT[:3, pos:pos + ne] = ea[eidx].T
            src_ids[pos:pos + ne] = gid_of_node[src[eidx]]
            dst_rel[pos:pos + ne] = perms[k][dst[eidx] - n_lo[k]] - b * 128
            pos += cap
        assert pos == EPC_PAD

        srcg = wrap16(src_ids)

        # onehots: oh1[p=edge-in-tile, t*128 + node] for scatter lhsT;
        #          oh2[p=node, t*128 + edge-in-tile] for dst-part lhsT
        # int8: one-hots are exactly representable; quarters the DRAM
        # stream (expanded to f32 on-chip by the Scalar engine)
        oh1 = np.zeros((128, T * 128), np.int8)
        oh2 = np.zeros((128, T * 128), np.int8)
        tt = np.arange(EPC_PAD) // 128
        pp = np.arange(EPC_PAD) % 128
        valid = dst_rel >= 0
        oh1[pp[valid], tt[valid] * 128 + dst_rel[valid]] = 1
        oh2[dst_rel[valid], tt[valid] * 128 + pp[valid]] = 1

        invp = np.full(NPC_PAD, -1, np.int64)
        invp[perms[k][:npc[k]]] = np.arange(npc[k])
        grel = np.full((128, NB), -1.0, np.float32)
        for b in range(NB):
            for p in range(128):
                orig = invp[b * 128 + p]
                if orig >= 0:
                    grel[p, b] = float(batch[n_lo[k] + orig] - g_lo[k])

        Gk = int(g_hi[k] - g_lo[k])
        invcnt = np.ones((128, 1), np.float32)
        slot_ids = np.zeros(G_MAX * 128, np.int64)
        for gl in range(G_MAX):
            g = g_lo[k] + gl
            if gl < Gk:
                nodes = np.arange(g_start[g], g_end[g])
                cnt = len(nodes)
                invcnt[gl, 0] = 1.0 / max(cnt, 1)
                sl = perms[k][nodes - n_lo[k]]
                slots = np.resize(sl, 128) if cnt > 0 else np.zeros(128, np.int64)
            else:
                slots = np.zeros(128, np.int64)
            slot_ids[gl * 128:(gl + 1) * 128] = slots
        slotg = wrap16(slot_ids)

        m = dict(shared)
        m.update(xT_own=xT_own, eaT=eaT, srcg=srcg, oh1=oh1, oh2=oh2,
                 grel=grel, invcnt=invcnt, slotg=slotg)
        in_maps.append(m)
        meta.append(dict(g_lo=int(g_lo[k]), g_hi=int(g_hi[k])))

    return in_maps, cfg, meta


# ----------------------------------------------------------------------------
# Bass program
# ----------------------------------------------------------------------------

def _build(cfg, debug_dump=False):
    NB = cfg["NB"]
    NPC_PAD = cfg["NPC_PAD"]
    NPAD_G = cfg["NPAD_G"]
    T = cfg["T"]
    EPC_PAD = cfg["EPC_PAD"]
    TPB = cfg["TPB"]
    G_MAX = cfg["G_MAX"]
    n_layers = cfg["n_layers"]
    CHUNKS = cfg["CHUNKS"]
    NCHUNK = len(CHUNKS)
    chunk_first = [0]
    for s in CHUNKS[:-1]:
        chunk_first.append(chunk_first[-1] + s)
    chunk_of_block = []
    for c, s in enumerate(CHUNKS):
        chunk_of_block += [c] * s
    table_base = [0]
    for s in CHUNKS:
        table_base.append(table_base[-1] + NCORES * s * 128)

    nc = bacc.Bacc("TRN2", debug=False, num_devices=NCORES)

    d_xT_own = nc.dram_tensor("xT_own", [11, NPC_PAD], F32, kind="ExternalInput")
    d_eaT = nc.dram_tensor("eaT", [4, EPC_PAD], F32, kind="ExternalInput")
    d_srcg = nc.dram_tensor("srcg", [128, EPC_PAD // 16], I16, kind="ExternalInput")
    d_oh1 = nc.dram_tensor("oh1", [128, T * 128], I8, kind="ExternalInput")
    d_oh2 = nc.dram_tensor("oh2", [128, T * 128], I8, kind="ExternalInput")
    d_grel = nc.dram_tensor("grel", [128, NB], F32, kind="ExternalInput")
    d_invcnt = nc.dram_tensor("invcnt", [128, 1], F32, kind="ExternalInput")
    d_slotg = nc.dram_tensor("slotg", [128, G_MAX * 128 // 16], I16, kind="ExternalInput")
    d_wnode = nc.dram_tensor("wnode", [11, H], F32, kind="ExternalInput")
    d_wedge = nc.dram_tensor("wedge", [4, H], F32, kind="ExternalInput")
    d_wfs = nc.dram_tensor("wfs", [128, n_layers * 768], F32, kind="ExternalInput")
    d_bfs = nc.dram_tensor("bfs", [1, n_layers * 256], F32, kind="ExternalInput")
    d_bns = nc.dram_tensor("bns", [128, n_layers], F32, kind="ExternalInput")
    d_bnb = nc.dram_tensor("bnb", [128, n_layers], F32, kind="ExternalInput")
    d_iota = nc.dram_tensor("iota", [128, 128], F32, kind="ExternalInput")
    d_w1p = nc.dram_tensor("w1p", [128, 3 * 256], F32, kind="ExternalInput")
    d_w2p = nc.dram_tensor("w2p", [128, 2 * 128], F32, kind="ExternalInput")
    d_w3p = nc.dram_tensor("w3p", [128, 64], F32, kind="ExternalInput")
    d_w4p = nc.dram_tensor("w4p", [64, 4], F32, kind="ExternalInput")
    d_hcol = nc.dram_tensor("hcol", [128, 8], F32, kind="ExternalInput")

    d_out4 = nc.dram_tensor("out4", [4, G_MAX], F32, kind="ExternalOutput")
    if debug_dump:
        d_hdump = nc.dram_tensor("hdump", [NPC_PAD, H], F32, kind="ExternalOutput")

    AF = mybir.ActivationFunctionType
    ALU = mybir.AluOpType

    with tile.TileContext(nc) as tc, nc.allow_low_precision(reason="bf16 conv path; PSUM accumulation stays f32"):
        import contextlib
        ctx = contextlib.ExitStack()
        with ctx:
            cpool = ctx.enter_context(tc.tile_pool(name="const", bufs=1))
            dram = ctx.enter_context(tc.tile_pool(name="dram", bufs=1, space="DRAM"))
            work = ctx.enter_context(tc.tile_pool(name="work", bufs=2))
            gbuf = ctx.enter_context(tc.tile_pool(name="gbuf", bufs=2))
            psum_p = ctx.enter_context(tc.tile_pool(name="psum_p", bufs=2, space="PSUM"))
            psum_a = ctx.enter_context(tc.tile_pool(name="psum_a", bufs=2, space="PSUM"))
            psum_t = ctx.enter_context(tc.tile_pool(name="psum_t", bufs=1, space="PSUM"))

            # DRAM state
            eT_d = dram.tile([128, EPC_PAD], F32, name="eT_d")
            # Per-chunk U staging. Separate tensors so each chunk's AllGather
            # depends only on its own blocks' U writes (collective overlaps
            # remaining compute).
            U_own_c = [dram.tile([CHUNKS[c] * 128, 256], F32, name=f"U_own{c}")
                       for c in range(NCHUNK)]
            # NOTE: plain (Local) DRAM, not addr_space="Shared" — the CoreSim
            # race detector enforces one writer instruction per Shared
            # tensor, which forbids the per-chunk AllGathers. Local-output
            # collectives bounce through per-collective internal buffers.
            U_tabs = []
            for i in range(n_layers):
                U_tabs.append(dram.tile([NPAD_G, 256], F32, name=f"U_tab{i}"))
            hin_slice = dram.tile([NPC_PAD, H], F32, name="hin_slice")

            # constants in SBUF
            ident = cpool.tile([128, 128], F32)
            make_identity(nc, ident[:])
            ones16 = cpool.tile([1, 128], F32)
            nc.vector.memset(ones16[:], 1.0)
            c_wnode = cpool.tile([11, H], F32)
            nc.sync.dma_start(out=c_wnode[:], in_=d_wnode[:])
            c_wedge = cpool.tile([4, H], F32)
            nc.sync.dma_start(out=c_wedge[:], in_=d_wedge[:])
            c_wfs = cpool.tile([128, n_layers * 768], F32)
            nc.sync.dma_start(out=c_wfs[:], in_=d_wfs[:])
            c_bfs = cpool.tile([1, n_layers * 256], F32)
            nc.sync.dma_start(out=c_bfs[:], in_=d_bfs[:])
            c_bns = cpool.tile([128, n_layers], F32)
            nc.sync.dma_start(out=c_bns[:], in_=d_bns[:])
            c_bnb = cpool.tile([128, n_layers], F32)
            nc.sync.dma_start(out=c_bnb[:], in_=d_bnb[:])
            c_iota = cpool.tile([128, 128], F32)
            nc.sync.dma_start(out=c_iota[:], in_=d_iota[:])
            c_srcg = cpool.tile([128, EPC_PAD // 16], I16)
            nc.sync.dma_start(out=c_srcg[:], in_=d_srcg[:])
            c_grel = cpool.tile([128, NB], F32)
            nc.sync.dma_start(out=c_grel[:], in_=d_grel[:])
            c_invcnt = cpool.tile([128, 1], F32)
            nc.sync.dma_start(out=c_invcnt[:], in_=d_invcnt[:])
            c_slotg = cpool.tile([128, G_MAX * 128 // 16], I16)
            nc.sync.dma_start(out=c_slotg[:], in_=d_slotg[:])
            c_w1p = cpool.tile([128, 3 * 256], F32)
            nc.sync.dma_start(out=c_w1p[:], in_=d_w1p[:])
            c_w2p = cpool.tile([128, 2 * 128], F32)
            nc.sync.dma_start(out=c_w2p[:], in_=d_w2p[:])
            c_w3p = cpool.tile([128, 64], F32)
            nc.sync.dma_start(out=c_w3p[:], in_=d_w3p[:])
            c_w4p = cpool.tile([64, 4], F32)
            nc.sync.dma_start(out=c_w4p[:], in_=d_w4p[:])
            c_hcol = cpool.tile([128, 8], F32)
            nc.sync.dma_start(out=c_hcol[:], in_=d_hcol[:])

            # persistent SBUF state.
            # h_own is FEATURE-major: [feat(p), block*128 + node]. This makes
            # h blocks directly usable as matmul lhsT for the U/V tables (no
            # per-block PE transpose + PSUM copy), and turns the BN
            # scale/shift into per-partition ACT scale/bias.
            h_own = cpool.tile([128, NPC_PAD], F32, name="h_own")
            # node-major copy of the final h (for pooling), written once.
            h_nm_all = cpool.tile([128, NPC_PAD], F32, name="h_nm_all")
            V_all = cpool.tile([128, NB * 256], F32, name="V_all")

            def emit_block_post(i_next, b):
                """After h_own block b is final: compute V (dst) and U (src)
                tables for layer i_next; DMA U slice to its chunk's staging
                buffer; when the chunk is complete, AllGather it into
                U_tabs[i_next] (overlapping remaining blocks' compute)."""
                bs_ = slice(b * 128, (b + 1) * 128)
                # V (dst part) | U (src part) in one N=512 matmul; bias rides
                # a K=1 accumulate into the U half. h_own is feature-major so
                # the h block IS the lhsT — no transpose needed.
                pall = psum_t.tile([128, 512], F32, tag="uv", bufs=2)
                nc.tensor.matmul(out=pall[:], lhsT=h_own[:, bs_],
                                 rhs=c_wfs[:, i_next * 768:i_next * 768 + 512],
                                 start=True, stop=False)
                nc.tensor.matmul(out=pall[:, 256:512], lhsT=ones16[:],
                                 rhs=c_bfs[:, i_next * 256:(i_next + 1) * 256],
                                 start=False, stop=True)
                nc.vector.tensor_copy(out=V_all[:, b * 256:(b + 1) * 256],
                                      in_=pall[:, 0:256])
                u32 = work.tile([128, 256], F32, tag="u32")
                nc.scalar.copy(out=u32[:], in_=pall[:, 256:512])
                c = chunk_of_block[b]
                boff = b - chunk_first[c]
                nc.sync.dma_start(
                    out=U_own_c[c][boff * 128:(boff + 1) * 128, :],
                    in_=u32[:])
                if b - chunk_first[c] == CHUNKS[c] - 1:
                    nc.gpsimd.collective_compute(
                        "AllGather", ALU.bypass,
                        replica_groups=[list(range(NCORES))],
                        ins=[U_own_c[c].opt()],
                        outs=[U_tabs[i_next][table_base[c]:table_base[c + 1], :].opt()])

            with tc.tile_pool(name="enc", bufs=1) as enc:
                # ---------------- encoder: own nodes (first, so the layer-0
                # U AllGather chunks launch ASAP and overlap the edge
                # encoder) ----------------
                for b in range(NB):
                    xo_sb = enc.tile([11, 128], F32, tag="xo_sb", bufs=2)
                    nc.sync.dma_start(out=xo_sb[:], in_=d_xT_own[:, b * 128:(b + 1) * 128])
                    ph = psum_a.tile([128, 128], F32, tag="agg")
                    # lhsT=W, rhs=x -> out [feat, node] (feature-major h)
                    nc.tensor.matmul(out=ph[:], lhsT=c_wnode[:],
                                     rhs=xo_sb[:], start=True, stop=True)
                    nc.scalar.activation(h_own[:, b * 128:(b + 1) * 128], ph[:], AF.Relu)
                    emit_block_post(0, b)

                # ---------------- encoder: edges ----------------
                # Triple-buffered: with bufs=1 the DMA->MM->relu->DMA chain
                # fully serialized (~3.5us/chunk, ~300us startup stall before
                # layer 0's first slab).
                for c in range(0, EPC_PAD, 256):
                    w = min(256, EPC_PAD - c)
                    ea_sb = enc.tile([4, 256], F32, tag="ea_sb", bufs=3)
                    nc.sync.dma_start(out=ea_sb[:, :w], in_=d_eaT[:, c:c + w])
                    pe = psum_p.tile([128, SLAB, 256], F32, tag="P")
                    nc.tensor.matmul(out=pe[:, 0, :w],
                                     lhsT=c_wedge[:], rhs=ea_sb[:, :w],
                                     start=True, stop=True)
                    et_sb = enc.tile([128, 256], F32, tag="et_sb", bufs=2)
                    nc.scalar.activation(et_sb[:, :w], pe[:, 0, :w], AF.Relu)
                    nc.sync.dma_start(out=eT_d[:, c:c + w], in_=et_sb[:, :w])

            # tile index -> block
            tile_block = []
            for b in range(NB):
                tile_block += [b] * TPB[b]
            first_tile_of_block = {}
            last_tile_of_block = {}
            for t, b in enumerate(tile_block):
                if b not in first_tile_of_block:
                    first_tile_of_block[b] = t
                last_tile_of_block[b] = t

            n_gch = (EPC_PAD + GCH - 1) // GCH
            n_slab = (T + SLAB - 1) // SLAB

            def block_finish(i, b, agg):
                # agg is [feat, node] (scatter matmul emits feature-major);
                # BN scale/shift are per-feature = per-partition, fused into
                # the ACT Relu as scale/bias.
                bs_ = slice(b * 128, (b + 1) * 128)
                t0 = work.tile([128, 128], F32, tag="t0")
                nc.vector.tensor_tensor(out=t0[:], in0=agg[:], in1=h_own[:, bs_],
                                        op=ALU.add)
                if i % 2 == 1:
                    t3 = work.tile([128, 128], F32, tag="t3")
                    nc.scalar.activation(t3[:], t0[:], AF.Relu,
                                         bias=c_bnb[:, i:i + 1],
                                         scale=c_bns[:, i:i + 1])
                    nc.vector.tensor_tensor(out=h_own[:, bs_], in0=t3[:],
                                            in1=h_own[:, bs_], op=ALU.add)
                else:
                    nc.scalar.activation(h_own[:, bs_], t0[:], AF.Relu,
                                         bias=c_bnb[:, i:i + 1],
                                         scale=c_bns[:, i:i + 1])
                if i < n_layers - 1:
                    emit_block_post(i + 1, b)
                else:
                    # node-major copy for pooling + the maxpool slot gather.
                    # (psum_a "agg" tag: only live at the last layer, so it
                    # shares banks with the scatter aggregators instead of
                    # costing psum_t a dedicated bank.)
                    tp = psum_a.tile([128, 128], F32, tag="agg")
                    nc.tensor.transpose(out=tp[:], in_=h_own[:, bs_],
                                        identity=ident[:])
                    nc.scalar.copy(out=h_nm_all[:, bs_], in_=tp[:])
                    nc.sync.dma_start(out=hin_slice[b * 128:(b + 1) * 128, :],
                                      in_=h_nm_all[:, bs_])

            # ---------------- conv layers ----------------
            for i in range(n_layers):
                u_t = []
                eTb_t = []
                oh1_t = []
                oh2_t = []
                for c in range(n_gch):
                    lo = c * GCH
                    hi = min(EPC_PAD, lo + GCH)
                    w = hi - lo
                    ug = gbuf.tile([128, GCH // 128, 256], F32, tag="ug", bufs=3)
                    nc.gpsimd.dma_gather(
                        out_ap=ug[:, :w // 128, :], in_ap=U_tabs[i][:],
                        idxs_ap=c_srcg[:, lo // 16:hi // 16],
                        num_idxs=w, num_idxs_reg=w, elem_size=256)
                    eTb = gbuf.tile([128, GCH], F32, tag="eTb", bufs=2)
                    nc.sync.dma_start(out=eTb[:, :w], in_=eT_d[:, lo:hi])
                    # one-hots stream as int8 (4x less HBM traffic — the DMA
                    # system saturates mid-layer otherwise) and expand to f32
                    # on the Scalar engine (own SBUF port, spare capacity)
                    oh1b8 = gbuf.tile([128, GCH], I8, tag="oh1b8", bufs=2)
                    nc.sync.dma_start(out=oh1b8[:, :w], in_=d_oh1[:, lo:hi])
                    oh1b = gbuf.tile([128, GCH], F32, tag="oh1b", bufs=2)
                    nc.scalar.copy(out=oh1b[:, :w], in_=oh1b8[:, :w])
                    oh2b8 = gbuf.tile([128, GCH], I8, tag="oh2b8", bufs=2)
                    nc.sync.dma_start(out=oh2b8[:, :w], in_=d_oh2[:, lo:hi])
                    oh2b = gbuf.tile([128, GCH], F32, tag="oh2b", bufs=2)
                    nc.scalar.copy(out=oh2b[:, :w], in_=oh2b8[:, :w])
                    u_t.append(ug)
                    eTb_t.append(eTb)
                    oh1_t.append(oh1b)
                    oh2_t.append(oh2b)

                wcol = i * 768
                pend = []  # (tiles, MSG tile) awaiting scatter

                def emit_scatter(tiles, MSG, aggs, i=i):
                    for j, t in enumerate(tiles):
                        b = tile_block[t]
                        ch, off = t * 128 // GCH, (t * 128 % GCH) // 128
                        if t == first_tile_of_block[b]:
                            aggs[b] = psum_a.tile([128, 128], F32, tag="agg",
                                                  name="agg")
                        # lhsT=MSG, rhs=onehot -> agg [feat, node]: same
                        # products/accumulation order as the node-major form
                        # (contraction over the same 128 edges), but the
                        # output lands feature-major for free.
                        nc.tensor.matmul(
                            out=aggs[b][:],
                            lhsT=MSG[:, j * 128:(j + 1) * 128],
                            rhs=oh1_t[ch][:, off * 128:(off + 1) * 128],
                            start=(t == first_tile_of_block[b]),
                            stop=(t == last_tile_of_block[b]))
                        if t == last_tile_of_block[b]:
                            block_finish(i, b, aggs.pop(b))

                aggs = {}
                for s in range(n_slab):
                    t0_ = s * SLAB
                    tiles = list(range(t0_, min(T, t0_ + SLAB)))
                    nj = len(tiles)
                    P = psum_p.tile([128, SLAB, 256], F32, tag="P")
                    for j, t in enumerate(tiles):
                        b = tile_block[t]
                        ch, off = t * 128 // GCH, (t * 128 % GCH) // 128
                        nc.tensor.matmul(out=P[:, j, :],
                                         lhsT=oh2_t[ch][:, off * 128:(off + 1) * 128],
                                         rhs=V_all[:, b * 256:(b + 1) * 256],
                                         start=True, stop=False)
                        nc.tensor.matmul(out=P[:, j, :],
                                         lhsT=eTb_t[ch][:, off * 128:(off + 1) * 128],
                                         rhs=c_wfs[:, wcol + 512:wcol + 768],
                                         start=False, stop=True)
                    ch0 = t0_ * 128 // GCH
                    off0 = (t0_ * 128 % GCH) // 128
                    w1 = nj * 128

                    # FS de-interleaved: FS[:,0,:]=-f, FS[:,1,:]=s — two half
                    # adds pay the strided PSUM/u read once so every later
                    # elementwise op runs on contiguous SBUF. f32: exp args
                    # must not be bf16-rounded — values reach +-40k.
                    # Fresh-output tiles for the tensor_scalar ops; fused
                    # 2-wide Exp (instruction count beats per-op density).
                    FS = work.tile([128, 2, SLAB * 128], F32, tag="FS")
                    nc.vector.tensor_tensor(
                        out=FS[:, 0, :w1], in0=P[:, :nj, 0:128],
                        in1=u_t[ch0][:, off0:off0 + nj, 0:128], op=ALU.add)
                    nc.vector.tensor_tensor(
                        out=FS[:, 1, :w1], in0=P[:, :nj, 128:256],
                        in1=u_t[ch0][:, off0:off0 + nj, 128:256], op=ALU.add)
                    # RS = max(s, 0) on the (less busy) Scalar engine
                    RS = work.tile([128, SLAB * 128], F32, tag="RS")
                    nc.scalar.activation(RS[:, :w1], FS[:, 1, :w1], AF.Relu)
                    # G[:,0] = clamp(-f, +-30); G[:,1] = max(-|s|, -30)
                    # (Exp LUT yields NaN/garbage for out-of-range args)
                    G = work.tile([128, 2, SLAB * 128], F32, tag="G")
                    nc.vector.tensor_scalar(
                        out=G[:, 0, :w1], in0=FS[:, 0, :w1],
                        scalar1=-CLAMP, scalar2=CLAMP, op0=ALU.max, op1=ALU.min)
                    T1 = work.tile([128, SLAB * 128], F32, tag="T1", bufs=1)
                    nc.vector.scalar_tensor_tensor(
                        out=T1[:, :w1], in0=FS[:, 1, :w1], scalar=0.0,
                        in1=RS[:, :w1], op0=ALU.min, op1=ALU.subtract)
                    nc.vector.tensor_scalar_max(out=G[:, 1, :w1],
                                                in0=T1[:, :w1], scalar1=-CLAMP)
                    # E = exp(G) (one fused 2-region pass); L = ln(1+E) split
                    # so the sigmoid path stays f32: L_f = ln(1+e^-f) = -ln(sig)
                    E = work.tile([128, 2, SLAB * 128], F32, tag="E", bufs=1)
                    nc.scalar.activation(E[:, :, :w1], G[:, :, :w1], AF.Exp)
                    LF = work.tile([128, SLAB * 128], F32, tag="LF", bufs=1)
                    nc.scalar.activation(LF[:, :w1], E[:, 0, :w1], AF.Ln, bias=1.0)
                    LS = work.tile([128, SLAB * 128], F32, tag="LS")
                    nc.scalar.activation(LS[:, :w1], E[:, 1, :w1], AF.Ln, bias=1.0)
                    SG = work.tile([128, SLAB * 128], F32, tag="SG")
                    nc.scalar.activation(SG[:, :w1], LF[:, :w1], AF.Exp, scale=-1.0)
                    SP = work.tile([128, SLAB * 128], F32, tag="SP")
                    nc.vector.tensor_tensor(out=SP[:, :w1], in0=RS[:, :w1],
                                            in1=LS[:, :w1], op=ALU.add)
                    MSG = work.tile([128, SLAB * 128], F32, tag="MSG")
                    nc.vector.tensor_tensor(out=MSG[:, :w1], in0=SP[:, :w1],
                                            in1=SG[:, :w1], op=ALU.mult)
                    pend.append((tiles, MSG))
                    if len(pend) > 1:
                        emit_scatter(*pend.pop(0), aggs)
                while pend:
                    emit_scatter(*pend.pop(0), aggs)
                # (U exchange for the next layer is issued per-chunk inside
                # emit_block_post, overlapping the rest of this layer.)

            if debug_dump:
                for b in range(NB):
                    nc.sync.dma_start(out=d_hdump[b * 128:(b + 1) * 128, :],
                                      in_=h_nm_all[:, b * 128:(b + 1) * 128])

            # ---------------- pooling ----------------
            ppool = psum_a.tile([128, 128], F32, tag="agg")
            for b in range(NB):
                ohg = work.tile([128, 128], F32, tag="ohg")
                nc.vector.tensor_tensor(
                    out=ohg[:], in0=c_grel[:, b:b + 1].to_broadcast([128, 128]),
                    in1=c_iota[:], op=ALU.is_equal)
                nc.tensor.matmul(out=ppool[:], lhsT=ohg[:],
                                 rhs=h_nm_all[:, b * 128:(b + 1) * 128],
                                 start=(b == 0), stop=(b == NB - 1))
            sum_nm = work.tile([128, 128], F32, tag="sum_nm")
            nc.vector.tensor_copy(out=sum_nm[:], in_=ppool[:])
            mean_nm = work.tile([128, 128], F32, tag="mean_nm")
            nc.scalar.activation(mean_nm[:], ppool[:], AF.Identity, scale=c_invcnt[:])

            gT = cpool.tile([128, 3 * G_MAX], F32, name="gT")
            pt = psum_a.tile([128, 128], F32, tag="agg")
            nc.tensor.transpose(out=pt[:], in_=mean_nm[:], identity=ident[:])
            nc.scalar.copy(out=gT[:, 0:G_MAX], in_=pt[:, 0:G_MAX])
            pt2 = psum_a.tile([128, 128], F32, tag="agg")
            nc.tensor.transpose(out=pt2[:], in_=sum_nm[:], identity=ident[:])
            nc.scalar.copy(out=gT[:, 2 * G_MAX:3 * G_MAX], in_=pt2[:, 0:G_MAX])

            # max pool via slot gather
            n_sch = (G_MAX * 128 + GCH - 1) // GCH
            gslot_t = []
            for c in range(n_sch):
                lo = c * GCH
                hi = min(G_MAX * 128, lo + GCH)
                w = hi - lo
                gslot = gbuf.tile([128, GCH // 128, H], F32, tag="gslot", bufs=1)
                nc.gpsimd.dma_gather(
                    out_ap=gslot[:, :w // 128, :], in_ap=hin_slice[:],
                    idxs_ap=c_slotg[:, lo // 16:hi // 16],
                    num_idxs=w, num_idxs_reg=w, elem_size=H)
                gslot_t.append(gslot)
            for g in range(G_MAX):
                ch, off = g * 128 // GCH, (g * 128 % GCH) // 128
                ptm = psum_a.tile([128, 128], F32, tag="agg")
                nc.tensor.transpose(out=ptm[:], in_=gslot_t[ch][:, off, :], identity=ident[:])
                nc.vector.reduce_max(out=gT[:, G_MAX + g:G_MAX + g + 1], in_=ptm[:],
                                     axis=mybir.AxisListType.X)

            # ---------------- heads ----------------
            p1a = psum_p.tile([128, SLAB, 256], F32, tag="P")
            p1b = psum_p.tile([128, SLAB, 256], F32, tag="P")
            for c in range(3):
                rhs = gT[:, c * G_MAX:(c + 1) * G_MAX]
                nc.tensor.matmul(out=p1a[:, 0, :G_MAX], lhsT=c_w1p[:, c * 256:c * 256 + 128],
                                 rhs=rhs, start=(c == 0), stop=(c == 2))
                nc.tensor.matmul(out=p1b[:, 0, :G_MAX], lhsT=c_w1p[:, c * 256 + 128:(c + 1) * 256],
                                 rhs=rhs, start=(c == 0), stop=(c == 2))
            g1a = work.tile([128, G_MAX], F32, tag="g1a")
            nc.scalar.activation(g1a[:], p1a[:, 0, :G_MAX], AF.Relu, bias=c_hcol[:, 2:3], scale=c_hcol[:, 0:1])
            g1b = work.tile([128, G_MAX], F32, tag="g1b")
            nc.scalar.activation(g1b[:], p1b[:, 0, :G_MAX], AF.Relu, bias=c_hcol[:, 3:4], scale=c_hcol[:, 1:2])

            p2 = psum_p.tile([128, SLAB, 256], F32, tag="P")
            nc.tensor.matmul(out=p2[:, 0, :G_MAX], lhsT=c_w2p[:, 0:128], rhs=g1a[:], start=True, stop=False)
            nc.tensor.matmul(out=p2[:, 0, :G_MAX], lhsT=c_w2p[:, 128:256], rhs=g1b[:], start=False, stop=True)
            g2 = work.tile([128, G_MAX], F32, tag="g2")
            nc.scalar.activation(g2[:], p2[:, 0, :G_MAX], AF.Relu, bias=c_hcol[:, 5:6], scale=c_hcol[:, 4:5])

            p3 = psum_p.tile([128, SLAB, 256], F32, tag="P")
            nc.tensor.matmul(out=p3[:64, 0, :G_MAX], lhsT=c_w3p[:], rhs=g2[:], start=True, stop=True)
            g3 = work.tile([64, G_MAX], F32, tag="g3")
            nc.scalar.activation(g3[:], p3[:64, 0, :G_MAX], AF.Relu, bias=c_hcol[:64, 6:7])

            p4 = psum_p.tile([128, SLAB, 256], F32, tag="P")
            nc.tensor.matmul(out=p4[:4, 0, :G_MAX], lhsT=c_w4p[:], rhs=g3[:], start=True, stop=True)
            o4 = work.tile([4, G_MAX], F32, tag="o4")
            nc.scalar.activation(o4[:], p4[:4, 0, :G_MAX], AF.Identity, bias=c_hcol[:4, 7:8])
            nc.sync.dma_start(out=d_out4[:], in_=o4[:])

    nc.compile()
    return nc


# ----------------------------------------------------------------------------
# Entry point
# ----------------------------------------------------------------------------

_CACHE = {}


def kernel(trace=False, n_layers=NLAYERS, debug_dump=False, **inputs):
    in_maps, cfg, meta = _prepare(inputs, n_layers=n_layers)
    key = (tuple(sorted(cfg.items())), debug_dump)
    if key not in _CACHE:
        _CACHE[key] = _build(cfg, debug_dump=debug_dump)
    nc = _CACHE[key]

    res = run_bass_kernel_spmd(nc, in_maps, core_ids=list(range(NCORES)), trace=trace)

    outs = [np.zeros((NGRAPH, 1), np.float32) for _ in range(4)]
    for k in range(NCORES):
        g_lo, g_hi = meta[k]["g_lo"], meta[k]["g_hi"]
        o4 = res.results[k]["out4"]   # [4, G_MAX]
        for j in range(4):
            outs[j][g_lo:g_hi, 0] = o4[j, :g_hi - g_lo]
    kernel._last_res = res
    if debug_dump:
        kernel._last_hdump = [res.results[k]["hdump"] for k in range(NCORES)]
        kernel._last_cfg = cfg
    return tuple(outs)

